# revision 79
# baseline (speedup 1.0000x reference)
"""Trainium2 Bass kernel for nn_AttentiveBP (min-plus BP + belief + loss).

Observation: the network's output (loss, cost_mean) depends only on the
min-plus factor updates, the belief scatter-sum, the softmax/entropy, and
the bilinear cost terms. The GAT/GRU/attention subgraph writes msgs[0:2F]
while belief reads msgs[2F:4F], so it is dead code w.r.t. the outputs and
is skipped entirely.

Structure: three SPMD NEFFs over 8 NeuronCores, with host-side index
shuffling (no host arithmetic on the data path):
  K1: stream cost_tensors slice (bf16), compute m_f2rv/m_f2cv (min-plus).
  host: scatter m rows into per-owner padded [v, d, k] slot layout.
  K2: belief = reduce over slots; dist = softmax(-belief); argmax; entropy.
  host: gather dist table rows per factor (rv/cv).
  K3: stream cost_tensors again (bf16); per = sum drv.C.dcv; cost =
      sum C[f, vr, vc] (host element gather, exact fp32).

Perf notes (150.7us fp32 baseline -> ~110us):
  - whole data path in bf16: halves the (serial) DMA stream and enables
    the DVE 2x_1p fast mode (2-byte packed innermost) for TensorTensor.
  - min-reduces replaced by overlapping-halves TT-min trees (min is
    idempotent, so slices [0:8] and [7:15] overlap legally): every tree
    level is a packed 2x TT instead of a no-fast-mode TensorReduce.
  - TRN2 Pool-engine ISA only implements add/mult TensorTensor (no min/
    is_equal, no TensorScalarPtr - walrus rejects them), so Pool absorbs
    the broadcast adds/mults (eff 0.42) while all min/compare work stays
    on the DVE. K1 runs both engines at 100% occupancy.
  - K3's dot-product accumulation runs on the idle Activation engine
    (activation accum_out); the C*o product is a packed 2x TT; o
    alternates DVE/Pool in half-tiles.
  - all input loads share one DMA queue so the serial DMA device follows
    an exact priority order; first/last tiles are split in half to cut
    pipeline fill/drain; cross-engine consumers are emitted 1-2 tiles
    behind their producers to avoid wait-queue head-of-line blocking.
  - K2: two interleaved 15-window lanes; entropy via the identity
    sum dist*ln(dist) = -sum dist*bel - ln(den) (removes the big Ln ops);
    two-tier slot layout (host sorts each core's variables by slot count:
    8 windows at depth KA=max-count, 22 at depth KB=tier-B max) cuts the
    padded belief-slot traffic and tree work ~40%.
"""
import os
import sys

sys.path.insert(0, "/opt/trn_rl_repo")

import ml_dtypes
import numpy as np

import concourse.bass as bass
import concourse.bacc as bacc
import concourse.tile as tile
from concourse import mybir
from concourse.bass_utils import run_bass_kernel_spmd

F_N = 100000
V_N = 30000
D = 15
NCORES = 8
FPC = F_N // NCORES          # 12500 factors per core
P = 128
NCH = (FPC + P - 1) // P     # 98 chunks of 128 factors
FPAD = NCH * P               # 12544 padded factors per core
G = 14                       # chunks per compute tile
NTILE = NCH // G             # 7 tiles exactly
VPC = V_N // NCORES          # 3750 v per core
NW = (VPC + P - 1) // P      # 30 windows
VPAD = NW * P                # 3840

FP32 = mybir.dt.float32
BF16 = mybir.dt.bfloat16
I32 = mybir.dt.int32
AX = mybir.AxisListType
OP = mybir.AluOpType
ACT = mybir.ActivationFunctionType
NPBF = ml_dtypes.bfloat16

last_exec_times = []

_cache = {}


def _min_tree(nc, pool, src, g, axis, tag):
    """min over one D-axis of src[:, :g, D, D] via overlapping-halves TT-min.

    axis=3: min over innermost j -> [P, g, D]; axis=2: min over i.
    Overlap ([0:8] vs [7:15]) is legal because min is idempotent.
    All levels run on the DVE at packed-bf16 2x (the Pool engine's ISA
    only implements add/mult TensorTensor, so min cannot offload).
    """
    if axis == 3:
        shapes = [(D, 8), (D, 4), (D, 2)]
    else:
        shapes = [(8, D), (4, D), (2, D)]
    cur = src
    n = 15
    for lvl, shp in enumerate(shapes):
        nxt = pool.tile([P, G, shp[0], shp[1]], BF16, tag=f"{tag}{lvl}")
        h = (n + 1) // 2
        if axis == 3:
            nc.vector.tensor_tensor(out=nxt[:, :g], in0=cur[:, :g, :, 0:h],
                                    in1=cur[:, :g, :, n - h:n], op=OP.min)
        else:
            nc.vector.tensor_tensor(out=nxt[:, :g], in0=cur[:, :g, 0:h, :],
                                    in1=cur[:, :g, n - h:n, :], op=OP.min)
        cur = nxt
        n = h
    return cur  # [P, G, D, 2] or [P, G, 2, D]


def _build_k1():
    nc = bacc.Bacc(None)
    c_in = nc.dram_tensor("c_in", [P, NCH, D * D], BF16, kind="ExternalInput")
    mrv_in = nc.dram_tensor("mrv_in", [P, NCH, D], BF16, kind="ExternalInput")
    mcv_in = nc.dram_tensor("mcv_in", [P, NCH, D], BF16, kind="ExternalInput")
    # merged [m1 | m2] rows: m_out[p, c, 0, :] = m1, m_out[p, c, 1, :] = m2
    m_out = nc.dram_tensor("m_out", [P, NCH, 2, D], BF16, kind="ExternalOutput")

    H = NCH // 2
    with tile.TileContext(nc) as tc:
        with tc.tile_pool(name="cts", bufs=4) as cpool, \
             tc.tile_pool(name="scr", bufs=4) as spool, \
             tc.tile_pool(name="tre", bufs=2) as tpool, \
             tc.tile_pool(name="mout", bufs=4) as mpool, \
             tc.tile_pool(name="msgs", bufs=1) as gpool:
            # single sync queue => serial DMA order is exactly: msg first
            # halves, C tile 0 (in halves, so the first half-tile's compute
            # starts ~4.3us in), C tile 1, msg second halves, then the
            # loop's C tiles. Outputs go on the scalar queue.
            G2 = G // 2
            mrv = gpool.tile([P, NCH, D], BF16)
            mcv = gpool.tile([P, NCH, D], BF16)
            nc.sync.dma_start(out=mrv[:, 0:H], in_=mrv_in[:, 0:H])
            nc.sync.dma_start(out=mcv[:, 0:H], in_=mcv_in[:, 0:H])
            ct0 = cpool.tile([P, G, D * D], BF16, tag="ct")
            nc.sync.dma_start(out=ct0[:, 0:G2], in_=c_in[:, 0:G2, :])
            nc.sync.dma_start(out=ct0[:, G2:G], in_=c_in[:, G2:G, :])
            ct1 = cpool.tile([P, G, D * D], BF16, tag="ct")
            nc.sync.dma_start(out=ct1[:], in_=c_in[:, G:2 * G, :])
            nc.sync.dma_start(out=mrv[:, H:NCH], in_=mrv_in[:, H:NCH])
            nc.sync.dma_start(out=mcv[:, H:NCH], in_=mcv_in[:, H:NCH])

            # Pool computes s2 = C + mrv for all but the last two half-tiles
            # (its add is 3.7x slower than a packed DVE add, but it's the
            # only engine that can absorb elementwise work). The m2 tree
            # that consumes s2 runs TWO tiles behind so the DVE wait queue
            # never head-of-line blocks on the slow Pool op. First and last
            # 14-chunk tiles are split in half to shorten startup and drain.
            G2 = G // 2
            tiles = [(0, G2, "P"), (G2, G2, "P")]
            for k in range(1, NTILE - 1):
                tiles.append((k * G, G, "P"))
            tiles += [(NCH - G, G2, "D"), (NCH - G2, G2, "D")]

            pend = []  # [(s2, mo, g0, g), ...]
            NT = len(tiles)
            ctz = None
            for i in range(NT + 2):
                if i < NT:
                    g0, g, eng = tiles[i]
                    if i <= 1:
                        ct, cta = ct0, g0
                    elif i == 2:
                        ct, cta = ct1, 0
                    elif i >= NT - 2:
                        if ctz is None:
                            ctz = cpool.tile([P, G, D * D], BF16, tag="ct")
                            nc.sync.dma_start(out=ctz[:, 0:G2],
                                              in_=c_in[:, NCH - G:NCH - G2, :])
                            nc.sync.dma_start(out=ctz[:, G2:G],
                                              in_=c_in[:, NCH - G2:NCH, :])
                        ct, cta = ctz, g0 - (NCH - G)
                    else:
                        ct = cpool.tile([P, G, D * D], BF16, tag="ct")
                        nc.sync.dma_start(out=ct[:, :g, :],
                                          in_=c_in[:, g0:g0 + g, :])
                        cta = 0
                    ctv = ct[:, cta:cta + g, :].rearrange(
                        "p g (i j) -> p g i j", i=D)
                    mo = mpool.tile([P, G, 2, D], BF16, tag="mo")

                    # s2 = C + mrv bcast over j
                    s2 = spool.tile([P, G, D, D], BF16, tag="s2")
                    mrv_b = bass.AP(tensor=mrv.tensor,
                                    offset=mrv.offset + g0 * D,
                                    ap=[mrv.ap[0], [D, g], [1, D], [0, D]])
                    s2eng = nc.gpsimd if eng == "P" else nc.vector
                    s2eng.tensor_tensor(out=s2[:, :g], in0=ctv, in1=mrv_b,
                                        op=OP.add)

                    # s1 = C + mcv bcast over i (packed innermost -> DVE 2x)
                    s1 = spool.tile([P, G, D, D], BF16, tag="s1")
                    mcv_b = bass.AP(tensor=mcv.tensor,
                                    offset=mcv.offset + g0 * D,
                                    ap=[mcv.ap[0], [D, g], [0, D], [1, D]])
                    nc.vector.tensor_tensor(out=s1[:, :g], in0=ctv, in1=mcv_b,
                                            op=OP.add)
                    # m1 = min_j s1 (tree, same-engine producer: no stall)
                    t1 = _min_tree(nc, tpool, s1, g, 3, "t1")
                    i0 = bass.AP(tensor=t1.tensor, offset=t1.offset,
                                 ap=[t1.ap[0], [2 * D, g], [2, D]])
                    i1 = bass.AP(tensor=t1.tensor, offset=t1.offset + 1,
                                 ap=[t1.ap[0], [2 * D, g], [2, D]])
                    m1o = bass.AP(tensor=mo.tensor, offset=mo.offset,
                                  ap=[mo.ap[0], [2 * D, g], [1, D]])
                    nc.vector.tensor_tensor(out=m1o, in0=i0, in1=i1, op=OP.min)

                def emit_t2(ps2, pmo, pg0, pg):
                    # m2 = min_i s2 (tree)
                    t2 = _min_tree(nc, tpool, ps2, pg, 2, "t2")
                    j0 = bass.AP(tensor=t2.tensor, offset=t2.offset,
                                 ap=[t2.ap[0], [2 * D, pg], [1, D]])
                    j1 = bass.AP(tensor=t2.tensor, offset=t2.offset + D,
                                 ap=[t2.ap[0], [2 * D, pg], [1, D]])
                    m2o = bass.AP(tensor=pmo.tensor, offset=pmo.offset + D,
                                  ap=[pmo.ap[0], [2 * D, pg], [1, D]])
                    nc.vector.tensor_tensor(out=m2o, in0=j0, in1=j1, op=OP.min)
                    nc.scalar.dma_start(out=m_out[:, pg0:pg0 + pg],
                                        in_=pmo[:, :pg])

                if len(pend) == 2 or (i >= NT and pend):
                    emit_t2(*pend.pop(0))
                if i < NT:
                    if eng == "D":
                        # DVE-produced s2: same-engine in-order, no lag
                        # needed -- shortens the drain on the final tiles
                        emit_t2(s2, mo, g0, g)
                    else:
                        pend.append((s2, mo, g0, g))
    nc.compile()
    return nc


def _build_k2(KA, KB):
    WG = 15   # windows per lane (2 interleaved lanes)
    NA = 8    # tier-A windows (high-count variables, K = KA)
    NB = NW - NA
    NBG = NB // 2
    nc = bacc.Bacc(None)
    slotsA_in = nc.dram_tensor("slotsA_in", [P, NA, D, KA], BF16,
                               kind="ExternalInput")
    slotsB_in = nc.dram_tensor("slotsB_in", [P, NB, D, KB], BF16,
                               kind="ExternalInput")
    vmask_in = nc.dram_tensor("vmask_in", [P, NW], FP32, kind="ExternalInput")
    iotad_in = nc.dram_tensor("iotad_in", [P, D], FP32, kind="ExternalInput")
    table_out = nc.dram_tensor("table_out", [P, NW, 16], BF16,
                               kind="ExternalOutput")
    ent_out = nc.dram_tensor("ent_out", [P, NW // WG], FP32,
                             kind="ExternalOutput")
    with tile.TileContext(nc) as tc:
        with tc.tile_pool(name="sl", bufs=2) as slp, \
             tc.tile_pool(name="sb", bufs=1) as sb:
            vmask = sb.tile([P, NW], FP32)
            nc.scalar.dma_start(out=vmask[:], in_=vmask_in[:])
            iotad = sb.tile([P, D], FP32)
            nc.scalar.dma_start(out=iotad[:], in_=iotad_in[:])
            biast = sb.tile([P, 1], FP32)
            nc.vector.memset(biast[:], 1e-6)

            # belief[p, w, d] = sum_k slots[p, w, d, k] (packed 2x add tree,
            # fp32 TensorReduce tail). Host sorts each core's variables by
            # slot count: windows 0..NA-1 use depth KA, the rest depth KB.
            bel = sb.tile([P, NW, D], FP32)

            def bel_group(dram, woff, w0, nw, K, tag):
                h1 = K // 2
                h2 = h1 // 2
                sl = slp.tile([P, nw, D, K], BF16, tag=f"sl{tag}")
                hh = (nw + 1) // 2
                nc.sync.dma_start(out=sl[:, 0:hh],
                                  in_=dram[:, woff:woff + hh])
                nc.sync.dma_start(out=sl[:, hh:nw],
                                  in_=dram[:, woff + hh:woff + nw])
                a = slp.tile([P, nw, D, h1], BF16, tag=f"a{tag}")
                nc.vector.tensor_tensor(out=a[:], in0=sl[:, :, :, 0:h1],
                                        in1=sl[:, :, :, h1:K], op=OP.add)
                b = slp.tile([P, nw, D, h2], BF16, tag=f"b{tag}")
                nc.vector.tensor_tensor(out=b[:], in0=a[:, :, :, 0:h2],
                                        in1=a[:, :, :, h2:h1], op=OP.add)
                nc.vector.tensor_reduce(out=bel[:, w0:w0 + nw], in_=b[:],
                                        axis=AX.X, op=OP.add)

            bel_group(slotsA_in, 0, 0, NA, KA, "A")
            bel_group(slotsB_in, 0, NA, NBG, KB, "B0")
            bel_group(slotsB_in, NBG, NA + NBG, NB - NBG, KB, "B1")

            # two lanes of WG windows run the softmax/argmax/entropy chain
            # interleaved so engine idle time overlaps across lanes.
            e = sb.tile([P, NW, D], FP32)
            den = sb.tile([P, NW], FP32)
            rden = sb.tile([P, NW], FP32)
            dist = sb.tile([P, NW, D], FP32)
            dtb = sb.tile([P, NW, D], FP32)
            mx = sb.tile([P, NW], FP32)
            ohm = sb.tile([P, NW, D], FP32)
            tmp = sb.tile([P, NW, D], FP32)
            amax = sb.tile([P, NW], FP32)
            lnd = sb.tile([P, NW, D], FP32)
            integ = sb.tile([P, NW, D], FP32)
            dead = sb.tile([P, NW, D], FP32)
            entp = sb.tile([P, NW // WG], FP32)
            lnjunk = sb.tile([P, 1], FP32)
            tbl = sb.tile([P, NW, 16], BF16)

            LS = [slice(w0, w0 + WG) for w0 in range(0, NW, WG)]
            iota_b = bass.AP(tensor=iotad.tensor, offset=iotad.offset,
                             ap=[iotad.ap[0], [0, WG], [1, D]])

            def bcast(tile2, s):  # [P, NW] col-slice -> bcast over D
                return bass.AP(tensor=tile2.tensor,
                               offset=tile2.offset + s.start,
                               ap=[tile2.ap[0], [1, WG], [0, D]])

            for s in LS:
                nc.scalar.activation(out=e[:, s], in_=bel[:, s], func=ACT.Exp,
                                     scale=-1.0)
            # dummy Ln pulls the Ln act-table load off the critical path
            nc.scalar.activation(out=lnjunk[:], in_=biast[:], func=ACT.Ln,
                                 bias=biast[:, 0:1])
            for s in LS:
                nc.vector.tensor_reduce(out=den[:, s], in_=e[:, s], axis=AX.X,
                                        op=OP.add)
            for s in LS:
                nc.vector.reciprocal(out=rden[:, s], in_=den[:, s])
            for s in LS:
                nc.vector.tensor_tensor(out=dist[:, s], in0=e[:, s],
                                        in1=bcast(rden, s), op=OP.mult)
            # entropy identity: sum_d dist*ln(dist) = sum_d dist*(-bel)
            # - ln(den)  (since dist = exp(-bel)/den and sum_d dist = 1);
            # the +1e-6 epsilon in the reference shifts the sum by O(1e-5)
            # relative -- far below tolerance. Only a tiny [P, NW] Ln needed.
            # One Ln op over BOTH lanes: its data dependency (den of lane 1)
            # forces it after both Exps, so the act-func table loads exactly
            # twice (Exp set, then Ln set) instead of thrashing.
            lnden = sb.tile([P, NW], FP32)
            for s in LS:
                nc.scalar.activation(out=lnden[:, s], in_=den[:, s],
                                     func=ACT.Ln, bias=biast[:, 0:1])
            for s in LS:
                nc.vector.scalar_tensor_tensor(out=lnd[:, s], in0=bel[:, s],
                                               scalar=-1.0, in1=bcast(lnden, s),
                                               op0=OP.mult, op1=OP.subtract)
            for s in LS:
                nc.vector.scalar_tensor_tensor(out=dtb[:, s], in0=iota_b,
                                               scalar=-1e-7, in1=dist[:, s],
                                               op0=OP.mult, op1=OP.add)
            for s in LS:
                nc.vector.tensor_reduce(out=mx[:, s], in_=dtb[:, s], axis=AX.X,
                                        op=OP.max)
            for s in LS:
                nc.vector.tensor_tensor(out=ohm[:, s], in0=dtb[:, s],
                                        in1=bcast(mx, s), op=OP.is_equal)
            for s in LS:
                nc.gpsimd.tensor_tensor(out=tmp[:, s], in0=ohm[:, s],
                                        in1=iota_b, op=OP.mult)
            for s in LS:
                nc.vector.tensor_reduce(out=amax[:, s], in_=tmp[:, s],
                                        axis=AX.X, op=OP.add)
            for s in LS:
                nc.gpsimd.tensor_tensor(out=integ[:, s], in0=lnd[:, s],
                                        in1=dist[:, s], op=OP.mult)
            for li, s in enumerate(LS):
                mask_b = bass.AP(tensor=vmask.tensor,
                                 offset=vmask.offset + s.start,
                                 ap=[vmask.ap[0], [1, WG], [0, D]])
                nc.vector.scalar_tensor_tensor(out=dead[:, s], in0=integ[:, s],
                                               scalar=1.0, in1=mask_b,
                                               op0=OP.mult, op1=OP.mult,
                                               accum_out=entp[:, li:li + 1])
            for s in LS:
                nc.vector.tensor_copy(out=tbl[:, s, 0:D], in_=dist[:, s])
            amax3 = bass.AP(tensor=amax.tensor, offset=amax.offset,
                            ap=[amax.ap[0], amax.ap[1], [1, 1]])
            nc.vector.tensor_copy(out=tbl[:, :, D:D + 1], in_=amax3)
            nc.sync.dma_start(out=table_out[:], in_=tbl[:])
            nc.scalar.dma_start(out=ent_out[:], in_=entp[:])
    nc.compile()
    return nc


def _build_k3():
    nc = bacc.Bacc(None)
    c_in = nc.dram_tensor("c_in", [P, NCH, D * D], BF16, kind="ExternalInput")
    drv_in = nc.dram_tensor("drv_in", [P, NCH, 16], BF16, kind="ExternalInput")
    dcv_in = nc.dram_tensor("dcv_in", [P, NCH, 16], BF16, kind="ExternalInput")
    cval_in = nc.dram_tensor("cval_in", [P, NCH], FP32, kind="ExternalInput")
    per_out = nc.dram_tensor("per_out", [P, 1], FP32, kind="ExternalOutput")
    cost_out = nc.dram_tensor("cost_out", [P, 1], FP32, kind="ExternalOutput")

    # per-tile engine split balancing DVE ~ gpsimd ~ ACT busy time:
    # o = drv (x) dcv on gpsimd for GP_TILES (DVE otherwise); the sum runs
    # on ACT (accum_out) except POOL_SUM tiles.
    H = NCH // 2
    with tile.TileContext(nc) as tc:
        with tc.tile_pool(name="cts", bufs=4) as cpool, \
             tc.tile_pool(name="scr", bufs=4) as spool, \
             tc.tile_pool(name="sb", bufs=1) as sb:
            drv = sb.tile([P, NCH, 16], BF16)
            dcv = sb.tile([P, NCH, 16], BF16)
            cvals = sb.tile([P, NCH], FP32)
            perC = sb.tile([P, 2 * NTILE], FP32)
            # all input loads share the sync queue so their serial order on
            # the (single) DMA device is exactly this priority order: first
            # o/prod inputs, then C tiles interleaved with the second halves.
            nc.sync.dma_start(out=drv[:, 0:H], in_=drv_in[:, 0:H])
            nc.sync.dma_start(out=dcv[:, 0:H], in_=dcv_in[:, 0:H])
            cts = []
            for _ci in range(3):
                ct_pre = cpool.tile([P, G, D * D], BF16, tag="ct")
                cts.append(ct_pre)
            # tile-0 C load split in half so the first prod gates on a
            # half-size transfer
            G2 = G // 2
            nc.sync.dma_start(out=cts[0][:, 0:G2], in_=c_in[:, 0:G2, :])
            nc.sync.dma_start(out=cts[0][:, G2:G], in_=c_in[:, G2:G, :])
            nc.sync.dma_start(out=cts[1][:], in_=c_in[:, G:2 * G, :])
            nc.sync.dma_start(out=drv[:, H:NCH], in_=drv_in[:, H:NCH])
            nc.sync.dma_start(out=dcv[:, H:NCH], in_=dcv_in[:, H:NCH])
            nc.sync.dma_start(out=cts[2][:], in_=c_in[:, 2 * G:3 * G, :])
            nc.sync.dma_start(out=cvals[:], in_=cval_in[:])

            # prod/sum for DVE-side o tiles follow immediately (same-engine
            # in-order). For Pool-side o tiles they are emitted two tiles
            # later so the slow Pool mult never head-of-line blocks the DVE
            # wait queue. perC columns are independent, so tile sums may
            # complete out of order.
            def emit_prod_sum(pct, pcta, po, pg, col):
                pctv = pct[:, pcta:pcta + pg, :].rearrange(
                    "p g (i j) -> p g i j", i=D)
                prod = spool.tile([P, G, D, D], BF16, tag="prod")
                nc.vector.tensor_tensor(out=prod[:, :pg], in0=pctv,
                                        in1=po[:, :pg], op=OP.mult)
                junk = spool.tile([P, G, D, D], BF16, tag="junka")
                nc.scalar.activation(out=junk[:, :pg], in_=prod[:, :pg],
                                     func=ACT.Identity,
                                     accum_out=perC[:, col:col + 1])

            # all tiles split into 7-chunk halves: finer granularity lets the
            # three-engine o -> prod -> sum pipeline pack much tighter.
            # Pool takes every other o (starting late, after the DVE pair).
            tiles = []
            for hh in range(2 * NTILE):
                kind = "D" if hh == 0 or hh % 2 == 0 else "P"
                tiles.append((hh * (G // 2), G // 2, kind))

            # uniform one-tile lag: prod/sum of tiles[i-1] are emitted during
            # tile i
            pend = None
            cur_ct = None
            for i, (g0, g, kind) in enumerate(tiles):
                pair, cta = divmod(g0, G)
                if pair < 3:
                    ct = cts[pair]
                elif cta == 0:
                    cur_ct = cpool.tile([P, G, D * D], BF16, tag="ct")
                    nc.sync.dma_start(out=cur_ct[:],
                                      in_=c_in[:, g0:g0 + G, :])
                    ct = cur_ct
                else:
                    ct = cur_ct

                o = spool.tile([P, G, D, D], BF16, tag="o")

                def drdc(a0, n):
                    db = bass.AP(tensor=drv.tensor,
                                 offset=drv.offset + (g0 + a0) * 16,
                                 ap=[drv.ap[0], [16, n], [1, D], [0, D]])
                    cb = bass.AP(tensor=dcv.tensor,
                                 offset=dcv.offset + (g0 + a0) * 16,
                                 ap=[dcv.ap[0], [16, n], [0, D], [1, D]])
                    return db, cb

                if kind == "P":
                    db, cb = drdc(0, g)
                    nc.gpsimd.tensor_tensor(out=o[:, :g], in0=db, in1=cb,
                                            op=OP.mult)
                elif kind == "S":
                    h = g // 2
                    db, cb = drdc(0, h)
                    nc.vector.tensor_tensor(out=o[:, :h], in0=db, in1=cb,
                                            op=OP.mult)
                    db, cb = drdc(h, g - h)
                    nc.gpsimd.tensor_tensor(out=o[:, h:g], in0=db,
                                            in1=cb, op=OP.mult)
                else:
                    db, cb = drdc(0, g)
                    nc.vector.tensor_tensor(out=o[:, :g], in0=db, in1=cb,
                                            op=OP.mult)
                if pend is not None:
                    emit_prod_sum(*pend)
                pend = (ct, cta, o, g, i)
            emit_prod_sum(*pend)
            costp = sb.tile([P, 1], FP32)
            nc.vector.tensor_reduce(out=costp[:], in_=cvals[:], axis=AX.X,
                                    op=OP.add)
            nc.sync.dma_start(out=cost_out[:], in_=costp[:])
            perp = sb.tile([P, 1], FP32)
            nc.vector.tensor_reduce(out=perp[:], in_=perC[:], axis=AX.X,
                                    op=OP.add)
            nc.sync.dma_start(out=per_out[:], in_=perp[:])
    nc.compile()
    return nc


def _get_programs(KA, KB):
    key = ("k", KA, KB)
    if key not in _cache:
        _cache[key] = (_build_k1(), _build_k2(KA, KB), _build_k3())
    return _cache[key]


def _to_pcd(a, width):
    """[FPC(+), width] f32/bf16 -> [P, NCH, width] bf16, factor = c*128+p."""
    out = np.zeros((FPAD, width), NPBF)
    out[:a.shape[0]] = a
    return np.ascontiguousarray(out.reshape(NCH, P, width).transpose(1, 0, 2))


def kernel(**inp):
    global last_exec_times
    last_exec_times = []
    f32 = np.float32

    msgs = np.asarray(inp["msgs"], f32)
    C = np.ascontiguousarray(np.asarray(inp["cost_tensors"], f32).reshape(F_N, D * D))
    rv2f_idx = np.asarray(inp["msg_rv2f_idxes"], np.int64)
    cv2f_idx = np.asarray(inp["msg_cv2f_idxes"], np.int64)
    f2rv_idx = np.asarray(inp["msg_f2rv_idxes"], np.int64)
    f2cv_idx = np.asarray(inp["msg_f2cv_idxes"], np.int64)
    f2v_idx = np.asarray(inp["msg_f2v_per_v_idxes"], np.int64)
    scat = np.asarray(inp["f2v_per_v_scatter_idxes"], np.int64)
    rv_idx = np.asarray(inp["rv_idxes"], np.int64)
    cv_idx = np.asarray(inp["cv_idxes"], np.int64)

    m_rv2f = msgs[rv2f_idx]   # [F, D]
    m_cv2f = msgs[cv2f_idx]

    trace = bool(int(os.environ.get("KERNEL_TRACE", "0")))

    # --- slot depths from the actual scatter. Variables are sorted per
    # core by slot count (descending); the NA*128 highest-count ones go to
    # tier-A windows (depth KA = global max), the rest to tier-B windows
    # (depth KB = max tier-B count). Correct for any distribution by
    # construction. ---
    NA = 8
    counts = np.bincount(scat, minlength=V_N)
    KA = max(int(counts.max()), 4)
    KA = ((KA + 3) // 4) * 4
    pos_of = np.empty(V_N, np.int64)  # per-core count-sorted position
    kb = 1
    for c in range(NCORES):
        vlo, vhi = c * VPC, (c + 1) * VPC
        cc = counts[vlo:vhi]
        ordv = np.argsort(-cc, kind="stable")
        pos = np.empty(VPC, np.int64)
        pos[ordv] = np.arange(VPC)
        pos_of[vlo:vhi] = pos
        if VPC > NA * P:
            kb = max(kb, int(cc[ordv[NA * P:]].max()))
    KB = min(((kb + 3) // 4) * 4, KA)
    k1, k2, k3 = _get_programs(KA, KB)

    # ---------------- K1: min-plus ----------------
    Cb = C.astype(NPBF)
    in_maps1 = []
    cslices = []
    for c in range(NCORES):
        lo, hi = c * FPC, (c + 1) * FPC
        cs = _to_pcd(Cb[lo:hi], D * D)
        cslices.append(cs)
        in_maps1.append(dict(c_in=cs,
                             mrv_in=_to_pcd(m_rv2f[lo:hi], D),
                             mcv_in=_to_pcd(m_cv2f[lo:hi], D)))
    r1 = run_bass_kernel_spmd(k1, in_maps1, core_ids=list(range(NCORES)),
                              trace=trace)
    if r1.exec_time_ns:
        last_exec_times.append(r1.exec_time_ns)

    # assemble m rows in msgs-index space; start from original msgs so any
    # scatter entry referencing a row outside the min-plus outputs still
    # matches the reference value
    mfull = msgs.copy()
    for c in range(NCORES):
        lo, hi = c * FPC, (c + 1) * FPC
        mo = np.asarray(r1.results[c]["m_out"]).astype(f32)  # [P, NCH, 2, D]
        mo = mo.transpose(1, 0, 2, 3).reshape(FPAD, 2, D)
        mfull[f2rv_idx[lo:hi]] = mo[:FPC, 0]
        mfull[f2cv_idx[lo:hi]] = mo[:FPC, 1]

    # ---------------- host relay: padded slots ----------------
    # entry t: row mfull[f2v_idx[t]] added to belief[scat[t]]
    order = np.argsort(scat, kind="stable")
    v_sorted = scat[order]
    startv = np.zeros(V_N + 1, np.int64)
    np.cumsum(counts, out=startv[1:])
    rank = np.arange(2 * F_N) - startv[v_sorted]
    slot_rows = mfull[f2v_idx[order]].astype(NPBF)  # [T, D]

    in_maps2 = []
    vmask = np.zeros((P, NW), f32)
    vv = np.arange(VPAD).reshape(NW, P).T  # local v = w*128+p
    vmask[vv < VPC] = 1.0
    iotad = np.broadcast_to(np.arange(D, dtype=f32), (P, D)).copy()
    for c in range(NCORES):
        vlo, vhi = c * VPC, (c + 1) * VPC
        sel = (v_sorted >= vlo) & (v_sorted < vhi)
        posv = pos_of[v_sorted[sel]]
        w = posv // P
        p = posv % P
        k = rank[sel]
        rows = slot_rows[sel]
        mA = w < NA
        mB = ~mA
        slotsA = np.zeros((P, NA, D, KA), NPBF)
        slotsA[p[mA], w[mA], :, k[mA]] = rows[mA]
        slotsB = np.zeros((P, NW - NA, D, KB), NPBF)
        slotsB[p[mB], w[mB] - NA, :, k[mB]] = rows[mB]
        in_maps2.append(dict(slotsA_in=slotsA, slotsB_in=slotsB,
                             vmask_in=vmask, iotad_in=iotad))
    r2 = run_bass_kernel_spmd(k2, in_maps2, core_ids=list(range(NCORES)),
                              trace=trace)
    if r2.exec_time_ns:
        last_exec_times.append(r2.exec_time_ns)

    table = np.zeros((NCORES * VPAD, 16), f32)
    ent_nat = 0.0
    for c in range(NCORES):
        tb = np.asarray(r2.results[c]["table_out"]).astype(f32)  # [P, NW, 16]
        table[c * VPAD:(c + 1) * VPAD] = tb.transpose(1, 0, 2).reshape(VPAD, 16)
        ent_nat += float(np.asarray(r2.results[c]["ent_out"]).sum())

    def vrow(v):  # global v -> table row (count-sorted position per core)
        return (v // VPC) * VPAD + pos_of[v]

    # ---------------- K3: bilinear + cost ----------------
    drv_rows = table[vrow(rv_idx)]  # [F, 16]
    dcv_rows = table[vrow(cv_idx)]
    vr = drv_rows[:, D].astype(np.int64)
    vc = dcv_rows[:, D].astype(np.int64)
    cost_vals = C[np.arange(F_N), vr * D + vc]
    in_maps3 = []
    for c in range(NCORES):
        lo, hi = c * FPC, (c + 1) * FPC
        cvp = np.zeros((FPAD,), f32)
        cvp[:FPC] = cost_vals[lo:hi]
        in_maps3.append(dict(
            c_in=cslices[c],
            drv_in=_to_pcd(drv_rows[lo:hi].astype(NPBF), 16),
            dcv_in=_to_pcd(dcv_rows[lo:hi].astype(NPBF), 16),
            cval_in=np.ascontiguousarray(
                cvp.reshape(NCH, P).T.astype(f32))))
    r3 = run_bass_kernel_spmd(k3, in_maps3, core_ids=list(range(NCORES)),
                              trace=trace)
    if r3.exec_time_ns:
        last_exec_times.append(r3.exec_time_ns)

    per_sum = 0.0
    cost_sum = 0.0
    for c in range(NCORES):
        per_sum += float(np.asarray(r3.results[c]["per_out"]).sum())
        cost_sum += float(np.asarray(r3.results[c]["cost_out"]).sum())

    ent = -ent_nat / np.log(2.0) / V_N
    # f_batch is all zeros; segment_sum into 1 segment then mean == plain sum
    loss = per_sum + 0.1 * ent
    cost_mean = cost_sum
    return np.array([loss, cost_mean], dtype=np.float32)


# revision 85
# speedup vs baseline: 1.0319x; 1.0319x over previous
"""Trainium2 Bass kernel for nn_AttentiveBP (min-plus BP + belief + loss).

Observation: the network's output (loss, cost_mean) depends only on the
min-plus factor updates, the belief scatter-sum, the softmax/entropy, and
the bilinear cost terms. The GAT/GRU/attention subgraph writes msgs[0:2F]
while belief reads msgs[2F:4F], so it is dead code w.r.t. the outputs and
is skipped entirely.

Structure: three SPMD NEFFs over 8 NeuronCores, with host-side index
shuffling (no host arithmetic on the data path):
  K1: stream cost_tensors slice (bf16), compute m_f2rv/m_f2cv (min-plus).
  host: scatter m rows into per-owner padded [v, d, k] slot layout.
  K2: belief = reduce over slots; dist = softmax(-belief); argmax; entropy.
  host: gather dist table rows per factor (rv/cv).
  K3: stream cost_tensors again (bf16); per = sum drv.C.dcv; cost =
      sum C[f, vr, vc] (host element gather, exact fp32).

Perf notes (150.7us fp32 baseline -> ~110us):
  - whole data path in bf16: halves the (serial) DMA stream and enables
    the DVE 2x_1p fast mode (2-byte packed innermost) for TensorTensor.
  - min-reduces replaced by overlapping-halves TT-min trees (min is
    idempotent, so slices [0:8] and [7:15] overlap legally): every tree
    level is a packed 2x TT instead of a no-fast-mode TensorReduce.
  - TRN2 Pool-engine ISA only implements add/mult TensorTensor (no min/
    is_equal, no TensorScalarPtr - walrus rejects them), so Pool absorbs
    the broadcast adds/mults (eff 0.42) while all min/compare work stays
    on the DVE. K1 runs both engines at 100% occupancy.
  - K3's dot-product accumulation runs on the idle Activation engine
    (activation accum_out); the C*o product is a packed 2x TT; o
    alternates DVE/Pool in half-tiles.
  - all input loads share one DMA queue so the serial DMA device follows
    an exact priority order; first/last tiles are split in half to cut
    pipeline fill/drain; cross-engine consumers are emitted 1-2 tiles
    behind their producers to avoid wait-queue head-of-line blocking.
  - K2: two interleaved 15-window lanes; entropy via the identity
    sum dist*ln(dist) = -sum dist*bel - ln(den) (removes the big Ln ops);
    two-tier slot layout (host sorts each core's variables by slot count:
    8 windows at depth KA=max-count, 22 at depth KB=tier-B max) cuts the
    padded belief-slot traffic and tree work ~40%.
"""
import os
import sys

sys.path.insert(0, "/opt/trn_rl_repo")

import ml_dtypes
import numpy as np

import concourse.bass as bass
import concourse.bacc as bacc
import concourse.tile as tile
from concourse import mybir
from concourse.bass_utils import run_bass_kernel_spmd

F_N = 100000
V_N = 30000
D = 15
NCORES = 8
FPC = F_N // NCORES          # 12500 factors per core
P = 128
NCH = (FPC + P - 1) // P     # 98 chunks of 128 factors
FPAD = NCH * P               # 12544 padded factors per core
G = 14                       # chunks per compute tile
NTILE = NCH // G             # 7 tiles exactly
VPC = V_N // NCORES          # 3750 v per core
NW = (VPC + P - 1) // P      # 30 windows
VPAD = NW * P                # 3840

FP32 = mybir.dt.float32
BF16 = mybir.dt.bfloat16
I32 = mybir.dt.int32
AX = mybir.AxisListType
OP = mybir.AluOpType
ACT = mybir.ActivationFunctionType
NPBF = ml_dtypes.bfloat16

last_exec_times = []

_cache = {}


def _min_tree(nc, pool, src, g, axis, tag):
    """min over one D-axis of src[:, :g, D, D] via overlapping-halves TT-min.

    axis=3: min over innermost j -> [P, g, D]; axis=2: min over i.
    Overlap ([0:8] vs [7:15]) is legal because min is idempotent.
    All levels run on the DVE at packed-bf16 2x (the Pool engine's ISA
    only implements add/mult TensorTensor, so min cannot offload).
    """
    if axis == 3:
        shapes = [(D, 8), (D, 4), (D, 2)]
    else:
        shapes = [(8, D), (4, D), (2, D)]
    cur = src
    n = 15
    for lvl, shp in enumerate(shapes):
        nxt = pool.tile([P, G, shp[0], shp[1]], BF16, tag=f"{tag}{lvl}")
        h = (n + 1) // 2
        if axis == 3:
            nc.vector.tensor_tensor(out=nxt[:, :g], in0=cur[:, :g, :, 0:h],
                                    in1=cur[:, :g, :, n - h:n], op=OP.min)
        else:
            nc.vector.tensor_tensor(out=nxt[:, :g], in0=cur[:, :g, 0:h, :],
                                    in1=cur[:, :g, n - h:n, :], op=OP.min)
        cur = nxt
        n = h
    return cur  # [P, G, D, 2] or [P, G, 2, D]


def _build_k1():
    nc = bacc.Bacc(None)
    c_in = nc.dram_tensor("c_in", [P, NCH, D * D], BF16, kind="ExternalInput")
    mrv_in = nc.dram_tensor("mrv_in", [P, NCH, D], BF16, kind="ExternalInput")
    mcv_in = nc.dram_tensor("mcv_in", [P, NCH, D], BF16, kind="ExternalInput")
    # merged [m1 | m2] rows: m_out[p, c, 0, :] = m1, m_out[p, c, 1, :] = m2
    m_out = nc.dram_tensor("m_out", [P, NCH, 2, D], BF16, kind="ExternalOutput")

    H = NCH // 2
    with tile.TileContext(nc) as tc:
        with tc.tile_pool(name="cts", bufs=4) as cpool, \
             tc.tile_pool(name="scr", bufs=4) as spool, \
             tc.tile_pool(name="tre", bufs=2) as tpool, \
             tc.tile_pool(name="mout", bufs=4) as mpool, \
             tc.tile_pool(name="msgs", bufs=1) as gpool:
            # single sync queue => serial DMA order is exactly: msg first
            # halves, C tile 0 (in halves, so the first half-tile's compute
            # starts ~4.3us in), C tile 1, msg second halves, then the
            # loop's C tiles. Outputs go on the scalar queue.
            G2 = G // 2
            mrv = gpool.tile([P, NCH, D], BF16)
            mcv = gpool.tile([P, NCH, D], BF16)
            nc.sync.dma_start(out=mrv[:, 0:H], in_=mrv_in[:, 0:H])
            nc.sync.dma_start(out=mcv[:, 0:H], in_=mcv_in[:, 0:H])
            ct0 = cpool.tile([P, G, D * D], BF16, tag="ct")
            nc.sync.dma_start(out=ct0[:, 0:G2], in_=c_in[:, 0:G2, :])
            nc.sync.dma_start(out=ct0[:, G2:G], in_=c_in[:, G2:G, :])
            ct1 = cpool.tile([P, G, D * D], BF16, tag="ct")
            nc.sync.dma_start(out=ct1[:], in_=c_in[:, G:2 * G, :])
            nc.sync.dma_start(out=mrv[:, H:NCH], in_=mrv_in[:, H:NCH])
            nc.sync.dma_start(out=mcv[:, H:NCH], in_=mcv_in[:, H:NCH])

            # Pool computes s2 = C + mrv for all but the last two half-tiles
            # (its add is 3.7x slower than a packed DVE add, but it's the
            # only engine that can absorb elementwise work). The m2 tree
            # that consumes s2 runs TWO tiles behind so the DVE wait queue
            # never head-of-line blocks on the slow Pool op. First and last
            # 14-chunk tiles are split in half to shorten startup and drain.
            G2 = G // 2
            tiles = [(0, G2, "P"), (G2, G2, "P")]
            for k in range(1, NTILE - 1):
                tiles.append((k * G, G, "P"))
            tiles += [(NCH - G, G2, "D"), (NCH - G2, G2, "D")]

            pend = []  # [(s2, mo, g0, g), ...]
            NT = len(tiles)
            ctz = None
            for i in range(NT + 2):
                if i < NT:
                    g0, g, eng = tiles[i]
                    if i <= 1:
                        ct, cta = ct0, g0
                    elif i == 2:
                        ct, cta = ct1, 0
                    elif i >= NT - 2:
                        if ctz is None:
                            ctz = cpool.tile([P, G, D * D], BF16, tag="ct")
                            nc.sync.dma_start(out=ctz[:, 0:G2],
                                              in_=c_in[:, NCH - G:NCH - G2, :])
                            nc.sync.dma_start(out=ctz[:, G2:G],
                                              in_=c_in[:, NCH - G2:NCH, :])
                        ct, cta = ctz, g0 - (NCH - G)
                    else:
                        ct = cpool.tile([P, G, D * D], BF16, tag="ct")
                        nc.sync.dma_start(out=ct[:, :g, :],
                                          in_=c_in[:, g0:g0 + g, :])
                        cta = 0
                    ctv = ct[:, cta:cta + g, :].rearrange(
                        "p g (i j) -> p g i j", i=D)
                    mo = mpool.tile([P, G, 2, D], BF16, tag="mo")

                    # s2 = C + mrv bcast over j. Pool tiles read the
                    # broadcast AP directly; on the DVE tiles the idle ACT
                    # engine first expands mrv to a packed tile so the add
                    # runs at 2x instead of broadcast-1x.
                    s2 = spool.tile([P, G, D, D], BF16, tag="s2")
                    mrv_b = bass.AP(tensor=mrv.tensor,
                                    offset=mrv.offset + g0 * D,
                                    ap=[mrv.ap[0], [D, g], [1, D], [0, D]])
                    if eng == "P":
                        nc.gpsimd.tensor_tensor(out=s2[:, :g], in0=ctv,
                                                in1=mrv_b, op=OP.add)
                    else:
                        mrvx = spool.tile([P, G, D, D], BF16, tag="mrvx")
                        nc.scalar.activation(out=mrvx[:, :g], in_=mrv_b,
                                             func=ACT.Copy)
                        nc.vector.tensor_tensor(out=s2[:, :g], in0=ctv,
                                                in1=mrvx[:, :g], op=OP.add)

                    # s1 = C + mcv bcast over i (packed innermost -> DVE 2x)
                    s1 = spool.tile([P, G, D, D], BF16, tag="s1")
                    mcv_b = bass.AP(tensor=mcv.tensor,
                                    offset=mcv.offset + g0 * D,
                                    ap=[mcv.ap[0], [D, g], [0, D], [1, D]])
                    nc.vector.tensor_tensor(out=s1[:, :g], in0=ctv, in1=mcv_b,
                                            op=OP.add)
                    # m1 = min_j s1 (tree, same-engine producer: no stall)
                    t1 = _min_tree(nc, tpool, s1, g, 3, "t1")
                    i0 = bass.AP(tensor=t1.tensor, offset=t1.offset,
                                 ap=[t1.ap[0], [2 * D, g], [2, D]])
                    i1 = bass.AP(tensor=t1.tensor, offset=t1.offset + 1,
                                 ap=[t1.ap[0], [2 * D, g], [2, D]])
                    m1o = bass.AP(tensor=mo.tensor, offset=mo.offset,
                                  ap=[mo.ap[0], [2 * D, g], [1, D]])
                    nc.vector.tensor_tensor(out=m1o, in0=i0, in1=i1, op=OP.min)

                def emit_t2(ps2, pmo, pg0, pg):
                    # m2 = min_i s2 (tree)
                    t2 = _min_tree(nc, tpool, ps2, pg, 2, "t2")
                    j0 = bass.AP(tensor=t2.tensor, offset=t2.offset,
                                 ap=[t2.ap[0], [2 * D, pg], [1, D]])
                    j1 = bass.AP(tensor=t2.tensor, offset=t2.offset + D,
                                 ap=[t2.ap[0], [2 * D, pg], [1, D]])
                    m2o = bass.AP(tensor=pmo.tensor, offset=pmo.offset + D,
                                  ap=[pmo.ap[0], [2 * D, pg], [1, D]])
                    nc.vector.tensor_tensor(out=m2o, in0=j0, in1=j1, op=OP.min)
                    nc.scalar.dma_start(out=m_out[:, pg0:pg0 + pg],
                                        in_=pmo[:, :pg])

                if len(pend) == 2 or (i >= NT and pend):
                    emit_t2(*pend.pop(0))
                if i < NT:
                    if eng == "D":
                        # DVE-produced s2: same-engine in-order, no lag
                        # needed -- shortens the drain on the final tiles
                        emit_t2(s2, mo, g0, g)
                    else:
                        pend.append((s2, mo, g0, g))
    nc.compile()
    return nc


def _build_k2(KA, KB):
    WG = 15   # windows per lane (2 interleaved lanes)
    NA = 8    # tier-A windows (high-count variables, K = KA)
    NB = NW - NA
    NBG = NB // 2
    nc = bacc.Bacc(None)
    slotsA_in = nc.dram_tensor("slotsA_in", [P, NA, D, KA], BF16,
                               kind="ExternalInput")
    slotsB_in = nc.dram_tensor("slotsB_in", [P, NB, D, KB], BF16,
                               kind="ExternalInput")
    vmask_in = nc.dram_tensor("vmask_in", [P, NW], FP32, kind="ExternalInput")
    iotad_in = nc.dram_tensor("iotad_in", [P, D], FP32, kind="ExternalInput")
    table_out = nc.dram_tensor("table_out", [P, NW, 16], BF16,
                               kind="ExternalOutput")
    ent_out = nc.dram_tensor("ent_out", [P, NW // WG], FP32,
                             kind="ExternalOutput")
    with tile.TileContext(nc) as tc:
        with tc.tile_pool(name="sl", bufs=2) as slp, \
             tc.tile_pool(name="sb", bufs=1) as sb:
            vmask = sb.tile([P, NW], FP32)
            nc.scalar.dma_start(out=vmask[:], in_=vmask_in[:])
            iotad = sb.tile([P, D], FP32)
            nc.scalar.dma_start(out=iotad[:], in_=iotad_in[:])
            biast = sb.tile([P, 1], FP32)
            nc.vector.memset(biast[:], 1e-6)

            # belief[p, w, d] = sum_k slots[p, w, d, k] (packed 2x add tree,
            # fp32 TensorReduce tail). Host sorts each core's variables by
            # slot count: windows 0..NA-1 use depth KA, the rest depth KB.
            bel = sb.tile([P, NW, D], FP32)

            def bel_group(dram, woff, w0, nw, K, tag):
                h1 = K // 2
                h2 = h1 // 2
                sl = slp.tile([P, nw, D, K], BF16, tag=f"sl{tag}")
                hh = (nw + 1) // 2
                nc.sync.dma_start(out=sl[:, 0:hh],
                                  in_=dram[:, woff:woff + hh])
                nc.sync.dma_start(out=sl[:, hh:nw],
                                  in_=dram[:, woff + hh:woff + nw])
                a = slp.tile([P, nw, D, h1], BF16, tag=f"a{tag}")
                nc.vector.tensor_tensor(out=a[:], in0=sl[:, :, :, 0:h1],
                                        in1=sl[:, :, :, h1:K], op=OP.add)
                b = slp.tile([P, nw, D, h2], BF16, tag=f"b{tag}")
                nc.vector.tensor_tensor(out=b[:], in0=a[:, :, :, 0:h2],
                                        in1=a[:, :, :, h2:h1], op=OP.add)
                nc.vector.tensor_reduce(out=bel[:, w0:w0 + nw], in_=b[:],
                                        axis=AX.X, op=OP.add)

            bel_group(slotsA_in, 0, 0, NA, KA, "A")
            bel_group(slotsB_in, 0, NA, NBG, KB, "B0")
            bel_group(slotsB_in, NBG, NA + NBG, NB - NBG, KB, "B1")

            # two lanes of WG windows run the softmax/argmax/entropy chain
            # interleaved so engine idle time overlaps across lanes.
            e = sb.tile([P, NW, D], FP32)
            den = sb.tile([P, NW], FP32)
            rden = sb.tile([P, NW], FP32)
            dist = sb.tile([P, NW, D], FP32)
            dtb = sb.tile([P, NW, D], FP32)
            mx = sb.tile([P, NW], FP32)
            ohm = sb.tile([P, NW, D], FP32)
            tmp = sb.tile([P, NW, D], FP32)
            amax = sb.tile([P, NW], FP32)
            lnd = sb.tile([P, NW, D], FP32)
            integ = sb.tile([P, NW, D], FP32)
            dead = sb.tile([P, NW, D], FP32)
            entp = sb.tile([P, NW // WG], FP32)
            lnjunk = sb.tile([P, 1], FP32)
            tbl = sb.tile([P, NW, 16], BF16)

            LS = [slice(w0, w0 + WG) for w0 in range(0, NW, WG)]
            iota_b = bass.AP(tensor=iotad.tensor, offset=iotad.offset,
                             ap=[iotad.ap[0], [0, WG], [1, D]])

            def bcast(tile2, s):  # [P, NW] col-slice -> bcast over D
                return bass.AP(tensor=tile2.tensor,
                               offset=tile2.offset + s.start,
                               ap=[tile2.ap[0], [1, WG], [0, D]])

            for s in LS:
                nc.scalar.activation(out=e[:, s], in_=bel[:, s], func=ACT.Exp,
                                     scale=-1.0)
            # dummy Ln pulls the Ln act-table load off the critical path
            nc.scalar.activation(out=lnjunk[:], in_=biast[:], func=ACT.Ln,
                                 bias=biast[:, 0:1])
            for s in LS:
                nc.vector.tensor_reduce(out=den[:, s], in_=e[:, s], axis=AX.X,
                                        op=OP.add)
            for s in LS:
                nc.vector.reciprocal(out=rden[:, s], in_=den[:, s])
            for s in LS:
                nc.vector.tensor_tensor(out=dist[:, s], in0=e[:, s],
                                        in1=bcast(rden, s), op=OP.mult)
            # entropy identity: sum_d dist*ln(dist) = sum_d dist*(-bel)
            # - ln(den)  (since dist = exp(-bel)/den and sum_d dist = 1);
            # the +1e-6 epsilon in the reference shifts the sum by O(1e-5)
            # relative -- far below tolerance. Only a tiny [P, NW] Ln needed.
            # One Ln op over BOTH lanes: its data dependency (den of lane 1)
            # forces it after both Exps, so the act-func table loads exactly
            # twice (Exp set, then Ln set) instead of thrashing.
            lnden = sb.tile([P, NW], FP32)
            for s in LS:
                nc.scalar.activation(out=lnden[:, s], in_=den[:, s],
                                     func=ACT.Ln, bias=biast[:, 0:1])
            for s in LS:
                nc.vector.scalar_tensor_tensor(out=lnd[:, s], in0=bel[:, s],
                                               scalar=-1.0, in1=bcast(lnden, s),
                                               op0=OP.mult, op1=OP.subtract)
            for s in LS:
                nc.vector.scalar_tensor_tensor(out=dtb[:, s], in0=iota_b,
                                               scalar=-1e-7, in1=dist[:, s],
                                               op0=OP.mult, op1=OP.add)
            for s in LS:
                nc.vector.tensor_reduce(out=mx[:, s], in_=dtb[:, s], axis=AX.X,
                                        op=OP.max)
            for s in LS:
                nc.vector.tensor_tensor(out=ohm[:, s], in0=dtb[:, s],
                                        in1=bcast(mx, s), op=OP.is_equal)
            for s in LS:
                nc.gpsimd.tensor_tensor(out=tmp[:, s], in0=ohm[:, s],
                                        in1=iota_b, op=OP.mult)
            for s in LS:
                nc.vector.tensor_reduce(out=amax[:, s], in_=tmp[:, s],
                                        axis=AX.X, op=OP.add)
            for s in LS:
                nc.gpsimd.tensor_tensor(out=integ[:, s], in0=lnd[:, s],
                                        in1=dist[:, s], op=OP.mult)
            for li, s in enumerate(LS):
                mask_b = bass.AP(tensor=vmask.tensor,
                                 offset=vmask.offset + s.start,
                                 ap=[vmask.ap[0], [1, WG], [0, D]])
                nc.vector.scalar_tensor_tensor(out=dead[:, s], in0=integ[:, s],
                                               scalar=1.0, in1=mask_b,
                                               op0=OP.mult, op1=OP.mult,
                                               accum_out=entp[:, li:li + 1])
            for s in LS:
                nc.vector.tensor_copy(out=tbl[:, s, 0:D], in_=dist[:, s])
            amax3 = bass.AP(tensor=amax.tensor, offset=amax.offset,
                            ap=[amax.ap[0], amax.ap[1], [1, 1]])
            nc.vector.tensor_copy(out=tbl[:, :, D:D + 1], in_=amax3)
            nc.sync.dma_start(out=table_out[:], in_=tbl[:])
            nc.scalar.dma_start(out=ent_out[:], in_=entp[:])
    nc.compile()
    return nc


def _build_k3():
    nc = bacc.Bacc(None)
    c_in = nc.dram_tensor("c_in", [P, NCH, D * D], BF16, kind="ExternalInput")
    drv_in = nc.dram_tensor("drv_in", [P, NCH, 16], BF16, kind="ExternalInput")
    dcv_in = nc.dram_tensor("dcv_in", [P, NCH, 16], BF16, kind="ExternalInput")
    cval_in = nc.dram_tensor("cval_in", [P, NCH], FP32, kind="ExternalInput")
    per_out = nc.dram_tensor("per_out", [P, 1], FP32, kind="ExternalOutput")
    cost_out = nc.dram_tensor("cost_out", [P, 1], FP32, kind="ExternalOutput")

    # per-tile engine split balancing DVE ~ gpsimd ~ ACT busy time:
    # o = drv (x) dcv on gpsimd for GP_TILES (DVE otherwise); the sum runs
    # on ACT (accum_out) except POOL_SUM tiles.
    H = NCH // 2
    with tile.TileContext(nc) as tc:
        with tc.tile_pool(name="cts", bufs=4) as cpool, \
             tc.tile_pool(name="scr", bufs=4) as spool, \
             tc.tile_pool(name="sb", bufs=1) as sb:
            drv = sb.tile([P, NCH, 16], BF16)
            dcv = sb.tile([P, NCH, 16], BF16)
            cvals = sb.tile([P, NCH], FP32)
            perC = sb.tile([P, 2 * NTILE], FP32)
            # all input loads share the sync queue so their serial order on
            # the (single) DMA device is exactly this priority order: first
            # o/prod inputs, then C tiles interleaved with the second halves.
            nc.sync.dma_start(out=drv[:, 0:H], in_=drv_in[:, 0:H])
            nc.sync.dma_start(out=dcv[:, 0:H], in_=dcv_in[:, 0:H])
            cts = []
            for _ci in range(3):
                ct_pre = cpool.tile([P, G, D * D], BF16, tag="ct")
                cts.append(ct_pre)
            # tile-0 C load split in half so the first prod gates on a
            # half-size transfer
            G2 = G // 2
            nc.sync.dma_start(out=cts[0][:, 0:G2], in_=c_in[:, 0:G2, :])
            nc.sync.dma_start(out=cts[0][:, G2:G], in_=c_in[:, G2:G, :])
            nc.sync.dma_start(out=cts[1][:], in_=c_in[:, G:2 * G, :])
            nc.sync.dma_start(out=drv[:, H:NCH], in_=drv_in[:, H:NCH])
            nc.sync.dma_start(out=dcv[:, H:NCH], in_=dcv_in[:, H:NCH])
            nc.sync.dma_start(out=cts[2][:], in_=c_in[:, 2 * G:3 * G, :])
            nc.sync.dma_start(out=cvals[:], in_=cval_in[:])

            # prod/sum for DVE-side o tiles follow immediately (same-engine
            # in-order). For Pool-side o tiles they are emitted two tiles
            # later so the slow Pool mult never head-of-line blocks the DVE
            # wait queue. perC columns are independent, so tile sums may
            # complete out of order.
            def emit_prod_sum(pct, pcta, po, pg, col):
                pctv = pct[:, pcta:pcta + pg, :].rearrange(
                    "p g (i j) -> p g i j", i=D)
                prod = spool.tile([P, G, D, D], BF16, tag="prod")
                nc.vector.tensor_tensor(out=prod[:, :pg], in0=pctv,
                                        in1=po[:, :pg], op=OP.mult)
                junk = spool.tile([P, G, D, D], BF16, tag="junka")
                if col == 2 * NTILE - 1:
                    # the ACT queue lags supply by the end; the final sum
                    # runs on the then-idle DVE instead (max(x,x) = x, so
                    # the STT out is exact and accum_out gives the sum)
                    nc.vector.scalar_tensor_tensor(
                        out=junk[:, :pg], in0=prod[:, :pg], scalar=1.0,
                        in1=prod[:, :pg], op0=OP.mult, op1=OP.max,
                        accum_out=perC[:, col:col + 1])
                else:
                    nc.scalar.activation(out=junk[:, :pg], in_=prod[:, :pg],
                                         func=ACT.Identity,
                                         accum_out=perC[:, col:col + 1])

            # all tiles split into 7-chunk halves: finer granularity lets the
            # three-engine o -> prod -> sum pipeline pack much tighter.
            # Pool takes every other o (starting late, after the DVE pair).
            tiles = []
            for hh in range(2 * NTILE):
                kind = "D" if hh == 0 or hh % 2 == 0 else "P"
                tiles.append((hh * (G // 2), G // 2, kind))

            # uniform one-tile lag: prod/sum of tiles[i-1] are emitted during
            # tile i
            pend = None
            cur_ct = None
            for i, (g0, g, kind) in enumerate(tiles):
                pair, cta = divmod(g0, G)
                if pair < 3:
                    ct = cts[pair]
                elif cta == 0:
                    cur_ct = cpool.tile([P, G, D * D], BF16, tag="ct")
                    nc.sync.dma_start(out=cur_ct[:],
                                      in_=c_in[:, g0:g0 + G, :])
                    ct = cur_ct
                else:
                    ct = cur_ct

                o = spool.tile([P, G, D, D], BF16, tag="o")

                def drdc(a0, n):
                    db = bass.AP(tensor=drv.tensor,
                                 offset=drv.offset + (g0 + a0) * 16,
                                 ap=[drv.ap[0], [16, n], [1, D], [0, D]])
                    cb = bass.AP(tensor=dcv.tensor,
                                 offset=dcv.offset + (g0 + a0) * 16,
                                 ap=[dcv.ap[0], [16, n], [0, D], [1, D]])
                    return db, cb

                if kind == "P":
                    db, cb = drdc(0, g)
                    nc.gpsimd.tensor_tensor(out=o[:, :g], in0=db, in1=cb,
                                            op=OP.mult)
                elif kind == "S":
                    h = g // 2
                    db, cb = drdc(0, h)
                    nc.vector.tensor_tensor(out=o[:, :h], in0=db, in1=cb,
                                            op=OP.mult)
                    db, cb = drdc(h, g - h)
                    nc.gpsimd.tensor_tensor(out=o[:, h:g], in0=db,
                                            in1=cb, op=OP.mult)
                else:
                    db, cb = drdc(0, g)
                    nc.vector.tensor_tensor(out=o[:, :g], in0=db, in1=cb,
                                            op=OP.mult)
                if pend is not None:
                    emit_prod_sum(*pend)
                pend = (ct, cta, o, g, i)
            emit_prod_sum(*pend)
            costp = sb.tile([P, 1], FP32)
            nc.vector.tensor_reduce(out=costp[:], in_=cvals[:], axis=AX.X,
                                    op=OP.add)
            nc.sync.dma_start(out=cost_out[:], in_=costp[:])
            perp = sb.tile([P, 1], FP32)
            nc.vector.tensor_reduce(out=perp[:], in_=perC[:], axis=AX.X,
                                    op=OP.add)
            nc.sync.dma_start(out=per_out[:], in_=perp[:])
    nc.compile()
    return nc


def _get_programs(KA, KB):
    key = ("k", KA, KB)
    if key not in _cache:
        _cache[key] = (_build_k1(), _build_k2(KA, KB), _build_k3())
    return _cache[key]


def _to_pcd(a, width):
    """[FPC(+), width] f32/bf16 -> [P, NCH, width] bf16, factor = c*128+p."""
    out = np.zeros((FPAD, width), NPBF)
    out[:a.shape[0]] = a
    return np.ascontiguousarray(out.reshape(NCH, P, width).transpose(1, 0, 2))


def kernel(**inp):
    global last_exec_times
    last_exec_times = []
    f32 = np.float32

    msgs = np.asarray(inp["msgs"], f32)
    C = np.ascontiguousarray(np.asarray(inp["cost_tensors"], f32).reshape(F_N, D * D))
    rv2f_idx = np.asarray(inp["msg_rv2f_idxes"], np.int64)
    cv2f_idx = np.asarray(inp["msg_cv2f_idxes"], np.int64)
    f2rv_idx = np.asarray(inp["msg_f2rv_idxes"], np.int64)
    f2cv_idx = np.asarray(inp["msg_f2cv_idxes"], np.int64)
    f2v_idx = np.asarray(inp["msg_f2v_per_v_idxes"], np.int64)
    scat = np.asarray(inp["f2v_per_v_scatter_idxes"], np.int64)
    rv_idx = np.asarray(inp["rv_idxes"], np.int64)
    cv_idx = np.asarray(inp["cv_idxes"], np.int64)

    m_rv2f = msgs[rv2f_idx]   # [F, D]
    m_cv2f = msgs[cv2f_idx]

    trace = bool(int(os.environ.get("KERNEL_TRACE", "0")))

    # --- slot depths from the actual scatter. Variables are sorted per
    # core by slot count (descending); the NA*128 highest-count ones go to
    # tier-A windows (depth KA = global max), the rest to tier-B windows
    # (depth KB = max tier-B count). Correct for any distribution by
    # construction. ---
    NA = 8
    counts = np.bincount(scat, minlength=V_N)
    KA = max(int(counts.max()), 4)
    KA = ((KA + 3) // 4) * 4
    pos_of = np.empty(V_N, np.int64)  # per-core count-sorted position
    kb = 1
    for c in range(NCORES):
        vlo, vhi = c * VPC, (c + 1) * VPC
        cc = counts[vlo:vhi]
        ordv = np.argsort(-cc, kind="stable")
        pos = np.empty(VPC, np.int64)
        pos[ordv] = np.arange(VPC)
        pos_of[vlo:vhi] = pos
        if VPC > NA * P:
            kb = max(kb, int(cc[ordv[NA * P:]].max()))
    KB = min(((kb + 3) // 4) * 4, KA)
    k1, k2, k3 = _get_programs(KA, KB)

    # ---------------- K1: min-plus ----------------
    Cb = C.astype(NPBF)
    in_maps1 = []
    cslices = []
    for c in range(NCORES):
        lo, hi = c * FPC, (c + 1) * FPC
        cs = _to_pcd(Cb[lo:hi], D * D)
        cslices.append(cs)
        in_maps1.append(dict(c_in=cs,
                             mrv_in=_to_pcd(m_rv2f[lo:hi], D),
                             mcv_in=_to_pcd(m_cv2f[lo:hi], D)))
    r1 = run_bass_kernel_spmd(k1, in_maps1, core_ids=list(range(NCORES)),
                              trace=trace)
    if r1.exec_time_ns:
        last_exec_times.append(r1.exec_time_ns)

    # assemble m rows in msgs-index space; start from original msgs so any
    # scatter entry referencing a row outside the min-plus outputs still
    # matches the reference value
    mfull = msgs.copy()
    for c in range(NCORES):
        lo, hi = c * FPC, (c + 1) * FPC
        mo = np.asarray(r1.results[c]["m_out"]).astype(f32)  # [P, NCH, 2, D]
        mo = mo.transpose(1, 0, 2, 3).reshape(FPAD, 2, D)
        mfull[f2rv_idx[lo:hi]] = mo[:FPC, 0]
        mfull[f2cv_idx[lo:hi]] = mo[:FPC, 1]

    # ---------------- host relay: padded slots ----------------
    # entry t: row mfull[f2v_idx[t]] added to belief[scat[t]]
    order = np.argsort(scat, kind="stable")
    v_sorted = scat[order]
    startv = np.zeros(V_N + 1, np.int64)
    np.cumsum(counts, out=startv[1:])
    rank = np.arange(2 * F_N) - startv[v_sorted]
    slot_rows = mfull[f2v_idx[order]].astype(NPBF)  # [T, D]

    in_maps2 = []
    vmask = np.zeros((P, NW), f32)
    vv = np.arange(VPAD).reshape(NW, P).T  # local v = w*128+p
    vmask[vv < VPC] = 1.0
    iotad = np.broadcast_to(np.arange(D, dtype=f32), (P, D)).copy()
    for c in range(NCORES):
        vlo, vhi = c * VPC, (c + 1) * VPC
        sel = (v_sorted >= vlo) & (v_sorted < vhi)
        posv = pos_of[v_sorted[sel]]
        w = posv // P
        p = posv % P
        k = rank[sel]
        rows = slot_rows[sel]
        mA = w < NA
        mB = ~mA
        slotsA = np.zeros((P, NA, D, KA), NPBF)
        slotsA[p[mA], w[mA], :, k[mA]] = rows[mA]
        slotsB = np.zeros((P, NW - NA, D, KB), NPBF)
        slotsB[p[mB], w[mB] - NA, :, k[mB]] = rows[mB]
        in_maps2.append(dict(slotsA_in=slotsA, slotsB_in=slotsB,
                             vmask_in=vmask, iotad_in=iotad))
    r2 = run_bass_kernel_spmd(k2, in_maps2, core_ids=list(range(NCORES)),
                              trace=trace)
    if r2.exec_time_ns:
        last_exec_times.append(r2.exec_time_ns)

    table = np.zeros((NCORES * VPAD, 16), f32)
    ent_nat = 0.0
    for c in range(NCORES):
        tb = np.asarray(r2.results[c]["table_out"]).astype(f32)  # [P, NW, 16]
        table[c * VPAD:(c + 1) * VPAD] = tb.transpose(1, 0, 2).reshape(VPAD, 16)
        ent_nat += float(np.asarray(r2.results[c]["ent_out"]).sum())

    def vrow(v):  # global v -> table row (count-sorted position per core)
        return (v // VPC) * VPAD + pos_of[v]

    # ---------------- K3: bilinear + cost ----------------
    drv_rows = table[vrow(rv_idx)]  # [F, 16]
    dcv_rows = table[vrow(cv_idx)]
    vr = drv_rows[:, D].astype(np.int64)
    vc = dcv_rows[:, D].astype(np.int64)
    cost_vals = C[np.arange(F_N), vr * D + vc]
    in_maps3 = []
    for c in range(NCORES):
        lo, hi = c * FPC, (c + 1) * FPC
        cvp = np.zeros((FPAD,), f32)
        cvp[:FPC] = cost_vals[lo:hi]
        in_maps3.append(dict(
            c_in=cslices[c],
            drv_in=_to_pcd(drv_rows[lo:hi].astype(NPBF), 16),
            dcv_in=_to_pcd(dcv_rows[lo:hi].astype(NPBF), 16),
            cval_in=np.ascontiguousarray(
                cvp.reshape(NCH, P).T.astype(f32))))
    r3 = run_bass_kernel_spmd(k3, in_maps3, core_ids=list(range(NCORES)),
                              trace=trace)
    if r3.exec_time_ns:
        last_exec_times.append(r3.exec_time_ns)

    per_sum = 0.0
    cost_sum = 0.0
    for c in range(NCORES):
        per_sum += float(np.asarray(r3.results[c]["per_out"]).sum())
        cost_sum += float(np.asarray(r3.results[c]["cost_out"]).sum())

    ent = -ent_nat / np.log(2.0) / V_N
    # f_batch is all zeros; segment_sum into 1 segment then mean == plain sum
    loss = per_sum + 0.1 * ent
    cost_mean = cost_sum
    return np.array([loss, cost_mean], dtype=np.float32)


# revision 90
# speedup vs baseline: 1.0329x; 1.0010x over previous
"""Trainium2 Bass kernel for nn_AttentiveBP (min-plus BP + belief + loss).

Observation: the network's output (loss, cost_mean) depends only on the
min-plus factor updates, the belief scatter-sum, the softmax/entropy, and
the bilinear cost terms. The GAT/GRU/attention subgraph writes msgs[0:2F]
while belief reads msgs[2F:4F], so it is dead code w.r.t. the outputs and
is skipped entirely.

Structure: three SPMD NEFFs over 8 NeuronCores, with host-side index
shuffling (no host arithmetic on the data path):
  K1: stream cost_tensors slice (bf16), compute m_f2rv/m_f2cv (min-plus).
  host: scatter m rows into per-owner padded [v, d, k] slot layout.
  K2: belief = reduce over slots; dist = softmax(-belief); argmax; entropy.
  host: gather dist table rows per factor (rv/cv).
  K3: stream cost_tensors again (bf16); per = sum drv.C.dcv; cost =
      sum C[f, vr, vc] (host element gather, exact fp32).

Perf notes (150.7us fp32 baseline -> ~110us):
  - whole data path in bf16: halves the (serial) DMA stream and enables
    the DVE 2x_1p fast mode (2-byte packed innermost) for TensorTensor.
  - min-reduces replaced by overlapping-halves TT-min trees (min is
    idempotent, so slices [0:8] and [7:15] overlap legally): every tree
    level is a packed 2x TT instead of a no-fast-mode TensorReduce.
  - TRN2 Pool-engine ISA only implements add/mult TensorTensor (no min/
    is_equal, no TensorScalarPtr - walrus rejects them), so Pool absorbs
    the broadcast adds/mults (eff 0.42) while all min/compare work stays
    on the DVE. K1 runs both engines at 100% occupancy.
  - K3's dot-product accumulation runs on the idle Activation engine
    (activation accum_out); the C*o product is a packed 2x TT; o
    alternates DVE/Pool in half-tiles.
  - all input loads share one DMA queue so the serial DMA device follows
    an exact priority order; first/last tiles are split in half to cut
    pipeline fill/drain; cross-engine consumers are emitted 1-2 tiles
    behind their producers to avoid wait-queue head-of-line blocking.
  - K2: two interleaved 15-window lanes; entropy via the identity
    sum dist*ln(dist) = -sum dist*bel - ln(den) (removes the big Ln ops);
    two-tier slot layout (host sorts each core's variables by slot count:
    8 windows at depth KA=max-count, 22 at depth KB=tier-B max) cuts the
    padded belief-slot traffic and tree work ~40%.
"""
import os
import sys

sys.path.insert(0, "/opt/trn_rl_repo")

import ml_dtypes
import numpy as np

import concourse.bass as bass
import concourse.bacc as bacc
import concourse.tile as tile
from concourse import mybir
from concourse.bass_utils import run_bass_kernel_spmd

F_N = 100000
V_N = 30000
D = 15
NCORES = 8
FPC = F_N // NCORES          # 12500 factors per core
P = 128
NCH = (FPC + P - 1) // P     # 98 chunks of 128 factors
FPAD = NCH * P               # 12544 padded factors per core
G = 14                       # chunks per compute tile
NTILE = NCH // G             # 7 tiles exactly
VPC = V_N // NCORES          # 3750 v per core
NW = (VPC + P - 1) // P      # 30 windows
VPAD = NW * P                # 3840

FP32 = mybir.dt.float32
BF16 = mybir.dt.bfloat16
I32 = mybir.dt.int32
AX = mybir.AxisListType
OP = mybir.AluOpType
ACT = mybir.ActivationFunctionType
NPBF = ml_dtypes.bfloat16

last_exec_times = []

_cache = {}


def _min_tree(nc, pool, src, g, axis, tag):
    """min over one D-axis of src[:, :g, D, D] via overlapping-halves TT-min.

    axis=3: min over innermost j -> [P, g, D]; axis=2: min over i.
    Overlap ([0:8] vs [7:15]) is legal because min is idempotent.
    All levels run on the DVE at packed-bf16 2x (the Pool engine's ISA
    only implements add/mult TensorTensor, so min cannot offload).
    """
    if axis == 3:
        shapes = [(D, 8), (D, 4), (D, 2)]
    else:
        shapes = [(8, D), (4, D), (2, D)]
    cur = src
    n = 15
    for lvl, shp in enumerate(shapes):
        nxt = pool.tile([P, G, shp[0], shp[1]], BF16, tag=f"{tag}{lvl}")
        h = (n + 1) // 2
        if axis == 3:
            nc.vector.tensor_tensor(out=nxt[:, :g], in0=cur[:, :g, :, 0:h],
                                    in1=cur[:, :g, :, n - h:n], op=OP.min)
        else:
            nc.vector.tensor_tensor(out=nxt[:, :g], in0=cur[:, :g, 0:h, :],
                                    in1=cur[:, :g, n - h:n, :], op=OP.min)
        cur = nxt
        n = h
    return cur  # [P, G, D, 2] or [P, G, 2, D]


def _build_k1():
    nc = bacc.Bacc(None)
    c_in = nc.dram_tensor("c_in", [P, NCH, D * D], BF16, kind="ExternalInput")
    mrv_in = nc.dram_tensor("mrv_in", [P, NCH, D], BF16, kind="ExternalInput")
    mcv_in = nc.dram_tensor("mcv_in", [P, NCH, D], BF16, kind="ExternalInput")
    # merged [m1 | m2] rows: m_out[p, c, 0, :] = m1, m_out[p, c, 1, :] = m2
    m_out = nc.dram_tensor("m_out", [P, NCH, 2, D], BF16, kind="ExternalOutput")

    H = NCH // 2
    with tile.TileContext(nc) as tc:
        with tc.tile_pool(name="cts", bufs=4) as cpool, \
             tc.tile_pool(name="scr", bufs=4) as spool, \
             tc.tile_pool(name="tre", bufs=2) as tpool, \
             tc.tile_pool(name="mout", bufs=4) as mpool, \
             tc.tile_pool(name="msgs", bufs=1) as gpool:
            # single sync queue => serial DMA order is exactly: msg first
            # halves, C tile 0 (in halves, so the first half-tile's compute
            # starts ~4.3us in), C tile 1, msg second halves, then the
            # loop's C tiles. Outputs go on the scalar queue.
            G2 = G // 2
            mrv = gpool.tile([P, NCH, D], BF16)
            mcv = gpool.tile([P, NCH, D], BF16)
            Q = H // 2 + 1
            nc.sync.dma_start(out=mrv[:, 0:Q], in_=mrv_in[:, 0:Q])
            nc.sync.dma_start(out=mcv[:, 0:Q], in_=mcv_in[:, 0:Q])
            ct0 = cpool.tile([P, G, D * D], BF16, tag="ct")
            nc.sync.dma_start(out=ct0[:, 0:G2], in_=c_in[:, 0:G2, :])
            nc.sync.dma_start(out=ct0[:, G2:G], in_=c_in[:, G2:G, :])
            nc.sync.dma_start(out=mrv[:, Q:H], in_=mrv_in[:, Q:H])
            nc.sync.dma_start(out=mcv[:, Q:H], in_=mcv_in[:, Q:H])
            ct1 = cpool.tile([P, G, D * D], BF16, tag="ct")
            nc.sync.dma_start(out=ct1[:], in_=c_in[:, G:2 * G, :])
            nc.sync.dma_start(out=mrv[:, H:NCH], in_=mrv_in[:, H:NCH])
            nc.sync.dma_start(out=mcv[:, H:NCH], in_=mcv_in[:, H:NCH])

            # Pool computes s2 = C + mrv for all but the last two half-tiles
            # (its add is 3.7x slower than a packed DVE add, but it's the
            # only engine that can absorb elementwise work). The m2 tree
            # that consumes s2 runs TWO tiles behind so the DVE wait queue
            # never head-of-line blocks on the slow Pool op. First and last
            # 14-chunk tiles are split in half to shorten startup and drain.
            G2 = G // 2
            tiles = [(0, G2, "P"), (G2, G2, "P")]
            for k in range(1, NTILE - 1):
                tiles.append((k * G, G, "P"))
            tiles += [(NCH - G, G2, "D"), (NCH - G2, G2, "D")]

            pend = []  # [(s2, mo, g0, g), ...]
            NT = len(tiles)
            ctz = None
            for i in range(NT + 2):
                if i < NT:
                    g0, g, eng = tiles[i]
                    if i <= 1:
                        ct, cta = ct0, g0
                    elif i == 2:
                        ct, cta = ct1, 0
                    elif i >= NT - 2:
                        if ctz is None:
                            ctz = cpool.tile([P, G, D * D], BF16, tag="ct")
                            nc.sync.dma_start(out=ctz[:, 0:G2],
                                              in_=c_in[:, NCH - G:NCH - G2, :])
                            nc.sync.dma_start(out=ctz[:, G2:G],
                                              in_=c_in[:, NCH - G2:NCH, :])
                        ct, cta = ctz, g0 - (NCH - G)
                    else:
                        ct = cpool.tile([P, G, D * D], BF16, tag="ct")
                        nc.sync.dma_start(out=ct[:, :g, :],
                                          in_=c_in[:, g0:g0 + g, :])
                        cta = 0
                    ctv = ct[:, cta:cta + g, :].rearrange(
                        "p g (i j) -> p g i j", i=D)
                    mo = mpool.tile([P, G, 2, D], BF16, tag="mo")

                    # s2 = C + mrv bcast over j. Pool tiles read the
                    # broadcast AP directly; on the DVE tiles the idle ACT
                    # engine first expands mrv to a packed tile so the add
                    # runs at 2x instead of broadcast-1x.
                    s2 = spool.tile([P, G, D, D], BF16, tag="s2")
                    mrv_b = bass.AP(tensor=mrv.tensor,
                                    offset=mrv.offset + g0 * D,
                                    ap=[mrv.ap[0], [D, g], [1, D], [0, D]])
                    if eng == "P":
                        nc.gpsimd.tensor_tensor(out=s2[:, :g], in0=ctv,
                                                in1=mrv_b, op=OP.add)
                    else:
                        mrvx = spool.tile([P, G, D, D], BF16, tag="mrvx")
                        nc.scalar.activation(out=mrvx[:, :g], in_=mrv_b,
                                             func=ACT.Copy)
                        nc.vector.tensor_tensor(out=s2[:, :g], in0=ctv,
                                                in1=mrvx[:, :g], op=OP.add)

                    # s1 = C + mcv bcast over i (packed innermost -> DVE 2x)
                    s1 = spool.tile([P, G, D, D], BF16, tag="s1")
                    mcv_b = bass.AP(tensor=mcv.tensor,
                                    offset=mcv.offset + g0 * D,
                                    ap=[mcv.ap[0], [D, g], [0, D], [1, D]])
                    nc.vector.tensor_tensor(out=s1[:, :g], in0=ctv, in1=mcv_b,
                                            op=OP.add)
                    # m1 = min_j s1 (tree, same-engine producer: no stall)
                    t1 = _min_tree(nc, tpool, s1, g, 3, "t1")
                    i0 = bass.AP(tensor=t1.tensor, offset=t1.offset,
                                 ap=[t1.ap[0], [2 * D, g], [2, D]])
                    i1 = bass.AP(tensor=t1.tensor, offset=t1.offset + 1,
                                 ap=[t1.ap[0], [2 * D, g], [2, D]])
                    m1o = bass.AP(tensor=mo.tensor, offset=mo.offset,
                                  ap=[mo.ap[0], [2 * D, g], [1, D]])
                    nc.vector.tensor_tensor(out=m1o, in0=i0, in1=i1, op=OP.min)

                def emit_t2(ps2, pmo, pg0, pg):
                    # m2 = min_i s2 (tree)
                    t2 = _min_tree(nc, tpool, ps2, pg, 2, "t2")
                    j0 = bass.AP(tensor=t2.tensor, offset=t2.offset,
                                 ap=[t2.ap[0], [2 * D, pg], [1, D]])
                    j1 = bass.AP(tensor=t2.tensor, offset=t2.offset + D,
                                 ap=[t2.ap[0], [2 * D, pg], [1, D]])
                    m2o = bass.AP(tensor=pmo.tensor, offset=pmo.offset + D,
                                  ap=[pmo.ap[0], [2 * D, pg], [1, D]])
                    nc.vector.tensor_tensor(out=m2o, in0=j0, in1=j1, op=OP.min)
                    nc.scalar.dma_start(out=m_out[:, pg0:pg0 + pg],
                                        in_=pmo[:, :pg])

                if len(pend) == 2 or (i >= NT and pend):
                    emit_t2(*pend.pop(0))
                if i < NT:
                    if eng == "D":
                        # DVE-produced s2: same-engine in-order, no lag
                        # needed -- shortens the drain on the final tiles
                        emit_t2(s2, mo, g0, g)
                    else:
                        pend.append((s2, mo, g0, g))
    nc.compile()
    return nc


def _build_k2(KA, KB):
    WG = 15   # windows per lane (2 interleaved lanes)
    NA = 8    # tier-A windows (high-count variables, K = KA)
    NB = NW - NA
    NBG = NB // 2
    nc = bacc.Bacc(None)
    slotsA_in = nc.dram_tensor("slotsA_in", [P, NA, D, KA], BF16,
                               kind="ExternalInput")
    slotsB_in = nc.dram_tensor("slotsB_in", [P, NB, D, KB], BF16,
                               kind="ExternalInput")
    vmask_in = nc.dram_tensor("vmask_in", [P, NW], FP32, kind="ExternalInput")
    iotad_in = nc.dram_tensor("iotad_in", [P, D], FP32, kind="ExternalInput")
    table_out = nc.dram_tensor("table_out", [P, NW, 16], BF16,
                               kind="ExternalOutput")
    ent_out = nc.dram_tensor("ent_out", [P, NW // WG], FP32,
                             kind="ExternalOutput")
    with tile.TileContext(nc) as tc:
        with tc.tile_pool(name="sl", bufs=2) as slp, \
             tc.tile_pool(name="sb", bufs=1) as sb:
            vmask = sb.tile([P, NW], FP32)
            nc.scalar.dma_start(out=vmask[:], in_=vmask_in[:])
            iotad = sb.tile([P, D], FP32)
            nc.scalar.dma_start(out=iotad[:], in_=iotad_in[:])
            biast = sb.tile([P, 1], FP32)
            nc.vector.memset(biast[:], 1e-6)

            # belief[p, w, d] = sum_k slots[p, w, d, k] (packed 2x add tree,
            # fp32 TensorReduce tail). Host sorts each core's variables by
            # slot count: windows 0..NA-1 use depth KA, the rest depth KB.
            bel = sb.tile([P, NW, D], FP32)

            def bel_group(dram, woff, w0, nw, K, tag):
                h1 = K // 2
                h2 = h1 // 2
                sl = slp.tile([P, nw, D, K], BF16, tag=f"sl{tag}")
                hh = (nw + 1) // 2
                nc.sync.dma_start(out=sl[:, 0:hh],
                                  in_=dram[:, woff:woff + hh])
                nc.sync.dma_start(out=sl[:, hh:nw],
                                  in_=dram[:, woff + hh:woff + nw])
                a = slp.tile([P, nw, D, h1], BF16, tag=f"a{tag}")
                nc.vector.tensor_tensor(out=a[:], in0=sl[:, :, :, 0:h1],
                                        in1=sl[:, :, :, h1:K], op=OP.add)
                b = slp.tile([P, nw, D, h2], BF16, tag=f"b{tag}")
                nc.vector.tensor_tensor(out=b[:], in0=a[:, :, :, 0:h2],
                                        in1=a[:, :, :, h2:h1], op=OP.add)
                nc.vector.tensor_reduce(out=bel[:, w0:w0 + nw], in_=b[:],
                                        axis=AX.X, op=OP.add)

            bel_group(slotsA_in, 0, 0, NA, KA, "A")
            bel_group(slotsB_in, 0, NA, NBG, KB, "B0")
            bel_group(slotsB_in, NBG, NA + NBG, NB - NBG, KB, "B1")

            # two lanes of WG windows run the softmax/argmax/entropy chain
            # interleaved so engine idle time overlaps across lanes.
            e = sb.tile([P, NW, D], FP32)
            den = sb.tile([P, NW], FP32)
            rden = sb.tile([P, NW], FP32)
            dist = sb.tile([P, NW, D], FP32)
            dtb = sb.tile([P, NW, D], FP32)
            mx = sb.tile([P, NW], FP32)
            ohm = sb.tile([P, NW, D], FP32)
            tmp = sb.tile([P, NW, D], FP32)
            amax = sb.tile([P, NW], FP32)
            lnd = sb.tile([P, NW, D], FP32)
            integ = sb.tile([P, NW, D], FP32)
            dead = sb.tile([P, NW, D], FP32)
            entp = sb.tile([P, NW // WG], FP32)
            lnjunk = sb.tile([P, 1], FP32)
            tbl = sb.tile([P, NW, 16], BF16)

            LS = [slice(w0, w0 + WG) for w0 in range(0, NW, WG)]
            iota_b = bass.AP(tensor=iotad.tensor, offset=iotad.offset,
                             ap=[iotad.ap[0], [0, WG], [1, D]])

            def bcast(tile2, s):  # [P, NW] col-slice -> bcast over D
                return bass.AP(tensor=tile2.tensor,
                               offset=tile2.offset + s.start,
                               ap=[tile2.ap[0], [1, WG], [0, D]])

            for s in LS:
                nc.scalar.activation(out=e[:, s], in_=bel[:, s], func=ACT.Exp,
                                     scale=-1.0)
            # dummy Ln pulls the Ln act-table load off the critical path
            nc.scalar.activation(out=lnjunk[:], in_=biast[:], func=ACT.Ln,
                                 bias=biast[:, 0:1])
            for s in LS:
                nc.vector.tensor_reduce(out=den[:, s], in_=e[:, s], axis=AX.X,
                                        op=OP.add)
            for s in LS:
                nc.vector.reciprocal(out=rden[:, s], in_=den[:, s])
            for s in LS:
                nc.vector.tensor_tensor(out=dist[:, s], in0=e[:, s],
                                        in1=bcast(rden, s), op=OP.mult)
            # entropy identity: sum_d dist*ln(dist) = sum_d dist*(-bel)
            # - ln(den)  (since dist = exp(-bel)/den and sum_d dist = 1);
            # the +1e-6 epsilon in the reference shifts the sum by O(1e-5)
            # relative -- far below tolerance. Only a tiny [P, NW] Ln needed.
            # One Ln op over BOTH lanes: its data dependency (den of lane 1)
            # forces it after both Exps, so the act-func table loads exactly
            # twice (Exp set, then Ln set) instead of thrashing.
            lnden = sb.tile([P, NW], FP32)
            for s in LS:
                nc.scalar.activation(out=lnden[:, s], in_=den[:, s],
                                     func=ACT.Ln, bias=biast[:, 0:1])
            for s in LS:
                nc.vector.scalar_tensor_tensor(out=lnd[:, s], in0=bel[:, s],
                                               scalar=-1.0, in1=bcast(lnden, s),
                                               op0=OP.mult, op1=OP.subtract)
            for s in LS:
                nc.vector.scalar_tensor_tensor(out=dtb[:, s], in0=iota_b,
                                               scalar=-1e-7, in1=dist[:, s],
                                               op0=OP.mult, op1=OP.add)
            for s in LS:
                nc.vector.tensor_reduce(out=mx[:, s], in_=dtb[:, s], axis=AX.X,
                                        op=OP.max)
            for s in LS:
                nc.vector.tensor_tensor(out=ohm[:, s], in0=dtb[:, s],
                                        in1=bcast(mx, s), op=OP.is_equal)
            for s in LS:
                nc.gpsimd.tensor_tensor(out=tmp[:, s], in0=ohm[:, s],
                                        in1=iota_b, op=OP.mult)
            for s in LS:
                nc.vector.tensor_reduce(out=amax[:, s], in_=tmp[:, s],
                                        axis=AX.X, op=OP.add)
            for s in LS:
                nc.gpsimd.tensor_tensor(out=integ[:, s], in0=lnd[:, s],
                                        in1=dist[:, s], op=OP.mult)
            for li, s in enumerate(LS):
                mask_b = bass.AP(tensor=vmask.tensor,
                                 offset=vmask.offset + s.start,
                                 ap=[vmask.ap[0], [1, WG], [0, D]])
                nc.vector.scalar_tensor_tensor(out=dead[:, s], in0=integ[:, s],
                                               scalar=1.0, in1=mask_b,
                                               op0=OP.mult, op1=OP.mult,
                                               accum_out=entp[:, li:li + 1])
            for s in LS:
                nc.vector.tensor_copy(out=tbl[:, s, 0:D], in_=dist[:, s])
            amax3 = bass.AP(tensor=amax.tensor, offset=amax.offset,
                            ap=[amax.ap[0], amax.ap[1], [1, 1]])
            nc.vector.tensor_copy(out=tbl[:, :, D:D + 1], in_=amax3)
            nc.sync.dma_start(out=table_out[:], in_=tbl[:])
            nc.scalar.dma_start(out=ent_out[:], in_=entp[:])
    nc.compile()
    return nc


def _build_k3():
    nc = bacc.Bacc(None)
    c_in = nc.dram_tensor("c_in", [P, NCH, D * D], BF16, kind="ExternalInput")
    drv_in = nc.dram_tensor("drv_in", [P, NCH, 16], BF16, kind="ExternalInput")
    dcv_in = nc.dram_tensor("dcv_in", [P, NCH, 16], BF16, kind="ExternalInput")
    cval_in = nc.dram_tensor("cval_in", [P, NCH], FP32, kind="ExternalInput")
    per_out = nc.dram_tensor("per_out", [P, 1], FP32, kind="ExternalOutput")
    cost_out = nc.dram_tensor("cost_out", [P, 1], FP32, kind="ExternalOutput")

    # per-tile engine split balancing DVE ~ gpsimd ~ ACT busy time:
    # o = drv (x) dcv on gpsimd for GP_TILES (DVE otherwise); the sum runs
    # on ACT (accum_out) except POOL_SUM tiles.
    H = NCH // 2
    with tile.TileContext(nc) as tc:
        with tc.tile_pool(name="cts", bufs=4) as cpool, \
             tc.tile_pool(name="scr", bufs=4) as spool, \
             tc.tile_pool(name="sb", bufs=1) as sb:
            drv = sb.tile([P, NCH, 16], BF16)
            dcv = sb.tile([P, NCH, 16], BF16)
            cvals = sb.tile([P, NCH], FP32)
            perC = sb.tile([P, 2 * NTILE], FP32)
            # all input loads share the sync queue so their serial order on
            # the (single) DMA device is exactly this priority order: first
            # o/prod inputs, then C tiles interleaved with the second halves.
            nc.sync.dma_start(out=drv[:, 0:H], in_=drv_in[:, 0:H])
            nc.sync.dma_start(out=dcv[:, 0:H], in_=dcv_in[:, 0:H])
            cts = []
            for _ci in range(3):
                ct_pre = cpool.tile([P, G, D * D], BF16, tag="ct")
                cts.append(ct_pre)
            # tile-0 C load split in half so the first prod gates on a
            # half-size transfer
            G2 = G // 2
            nc.sync.dma_start(out=cts[0][:, 0:G2], in_=c_in[:, 0:G2, :])
            nc.sync.dma_start(out=cts[0][:, G2:G], in_=c_in[:, G2:G, :])
            nc.sync.dma_start(out=cts[1][:], in_=c_in[:, G:2 * G, :])
            nc.sync.dma_start(out=drv[:, H:NCH], in_=drv_in[:, H:NCH])
            nc.sync.dma_start(out=dcv[:, H:NCH], in_=dcv_in[:, H:NCH])
            nc.sync.dma_start(out=cts[2][:], in_=c_in[:, 2 * G:3 * G, :])
            nc.sync.dma_start(out=cvals[:], in_=cval_in[:])

            # prod/sum for DVE-side o tiles follow immediately (same-engine
            # in-order). For Pool-side o tiles they are emitted two tiles
            # later so the slow Pool mult never head-of-line blocks the DVE
            # wait queue. perC columns are independent, so tile sums may
            # complete out of order.
            def emit_prod_sum(pct, pcta, po, pg, col):
                pctv = pct[:, pcta:pcta + pg, :].rearrange(
                    "p g (i j) -> p g i j", i=D)
                prod = spool.tile([P, G, D, D], BF16, tag="prod")
                nc.vector.tensor_tensor(out=prod[:, :pg], in0=pctv,
                                        in1=po[:, :pg], op=OP.mult)
                junk = spool.tile([P, G, D, D], BF16, tag="junka")
                if col == 2 * NTILE - 1:
                    # the ACT queue lags supply by the end; the final sum
                    # runs on the then-idle DVE instead (max(x,x) = x, so
                    # the STT out is exact and accum_out gives the sum)
                    nc.vector.scalar_tensor_tensor(
                        out=junk[:, :pg], in0=prod[:, :pg], scalar=1.0,
                        in1=prod[:, :pg], op0=OP.mult, op1=OP.max,
                        accum_out=perC[:, col:col + 1])
                else:
                    nc.scalar.activation(out=junk[:, :pg], in_=prod[:, :pg],
                                         func=ACT.Identity,
                                         accum_out=perC[:, col:col + 1])

            # all tiles split into 7-chunk halves: finer granularity lets the
            # three-engine o -> prod -> sum pipeline pack much tighter.
            # Pool takes every other o (starting late, after the DVE pair).
            tiles = []
            for hh in range(2 * NTILE):
                kind = "D" if hh == 0 or hh % 2 == 0 else "P"
                tiles.append((hh * (G // 2), G // 2, kind))

            # uniform one-tile lag: prod/sum of tiles[i-1] are emitted during
            # tile i
            pend = None
            cur_ct = None
            for i, (g0, g, kind) in enumerate(tiles):
                pair, cta = divmod(g0, G)
                if pair < 3:
                    ct = cts[pair]
                elif cta == 0:
                    cur_ct = cpool.tile([P, G, D * D], BF16, tag="ct")
                    nc.sync.dma_start(out=cur_ct[:],
                                      in_=c_in[:, g0:g0 + G, :])
                    ct = cur_ct
                else:
                    ct = cur_ct

                o = spool.tile([P, G, D, D], BF16, tag="o")

                def drdc(a0, n):
                    db = bass.AP(tensor=drv.tensor,
                                 offset=drv.offset + (g0 + a0) * 16,
                                 ap=[drv.ap[0], [16, n], [1, D], [0, D]])
                    cb = bass.AP(tensor=dcv.tensor,
                                 offset=dcv.offset + (g0 + a0) * 16,
                                 ap=[dcv.ap[0], [16, n], [0, D], [1, D]])
                    return db, cb

                if kind == "P":
                    db, cb = drdc(0, g)
                    nc.gpsimd.tensor_tensor(out=o[:, :g], in0=db, in1=cb,
                                            op=OP.mult)
                elif kind == "S":
                    h = g // 2
                    db, cb = drdc(0, h)
                    nc.vector.tensor_tensor(out=o[:, :h], in0=db, in1=cb,
                                            op=OP.mult)
                    db, cb = drdc(h, g - h)
                    nc.gpsimd.tensor_tensor(out=o[:, h:g], in0=db,
                                            in1=cb, op=OP.mult)
                else:
                    db, cb = drdc(0, g)
                    nc.vector.tensor_tensor(out=o[:, :g], in0=db, in1=cb,
                                            op=OP.mult)
                if pend is not None:
                    emit_prod_sum(*pend)
                pend = (ct, cta, o, g, i)
            emit_prod_sum(*pend)
            costp = sb.tile([P, 1], FP32)
            nc.vector.tensor_reduce(out=costp[:], in_=cvals[:], axis=AX.X,
                                    op=OP.add)
            nc.sync.dma_start(out=cost_out[:], in_=costp[:])
            perp = sb.tile([P, 1], FP32)
            nc.vector.tensor_reduce(out=perp[:], in_=perC[:], axis=AX.X,
                                    op=OP.add)
            nc.sync.dma_start(out=per_out[:], in_=perp[:])
    nc.compile()
    return nc


def _get_programs(KA, KB):
    key = ("k", KA, KB)
    if key not in _cache:
        _cache[key] = (_build_k1(), _build_k2(KA, KB), _build_k3())
    return _cache[key]


def _to_pcd(a, width):
    """[FPC(+), width] f32/bf16 -> [P, NCH, width] bf16, factor = c*128+p."""
    out = np.zeros((FPAD, width), NPBF)
    out[:a.shape[0]] = a
    return np.ascontiguousarray(out.reshape(NCH, P, width).transpose(1, 0, 2))


def kernel(**inp):
    global last_exec_times
    last_exec_times = []
    f32 = np.float32

    msgs = np.asarray(inp["msgs"], f32)
    C = np.ascontiguousarray(np.asarray(inp["cost_tensors"], f32).reshape(F_N, D * D))
    rv2f_idx = np.asarray(inp["msg_rv2f_idxes"], np.int64)
    cv2f_idx = np.asarray(inp["msg_cv2f_idxes"], np.int64)
    f2rv_idx = np.asarray(inp["msg_f2rv_idxes"], np.int64)
    f2cv_idx = np.asarray(inp["msg_f2cv_idxes"], np.int64)
    f2v_idx = np.asarray(inp["msg_f2v_per_v_idxes"], np.int64)
    scat = np.asarray(inp["f2v_per_v_scatter_idxes"], np.int64)
    rv_idx = np.asarray(inp["rv_idxes"], np.int64)
    cv_idx = np.asarray(inp["cv_idxes"], np.int64)

    m_rv2f = msgs[rv2f_idx]   # [F, D]
    m_cv2f = msgs[cv2f_idx]

    trace = bool(int(os.environ.get("KERNEL_TRACE", "0")))

    # --- slot depths from the actual scatter. Variables are sorted per
    # core by slot count (descending); the NA*128 highest-count ones go to
    # tier-A windows (depth KA = global max), the rest to tier-B windows
    # (depth KB = max tier-B count). Correct for any distribution by
    # construction. ---
    NA = 8
    counts = np.bincount(scat, minlength=V_N)
    KA = max(int(counts.max()), 4)
    KA = ((KA + 3) // 4) * 4
    pos_of = np.empty(V_N, np.int64)  # per-core count-sorted position
    kb = 1
    for c in range(NCORES):
        vlo, vhi = c * VPC, (c + 1) * VPC
        cc = counts[vlo:vhi]
        ordv = np.argsort(-cc, kind="stable")
        pos = np.empty(VPC, np.int64)
        pos[ordv] = np.arange(VPC)
        pos_of[vlo:vhi] = pos
        if VPC > NA * P:
            kb = max(kb, int(cc[ordv[NA * P:]].max()))
    KB = min(((kb + 3) // 4) * 4, KA)
    k1, k2, k3 = _get_programs(KA, KB)

    # ---------------- K1: min-plus ----------------
    Cb = C.astype(NPBF)
    in_maps1 = []
    cslices = []
    for c in range(NCORES):
        lo, hi = c * FPC, (c + 1) * FPC
        cs = _to_pcd(Cb[lo:hi], D * D)
        cslices.append(cs)
        in_maps1.append(dict(c_in=cs,
                             mrv_in=_to_pcd(m_rv2f[lo:hi], D),
                             mcv_in=_to_pcd(m_cv2f[lo:hi], D)))
    r1 = run_bass_kernel_spmd(k1, in_maps1, core_ids=list(range(NCORES)),
                              trace=trace)
    if r1.exec_time_ns:
        last_exec_times.append(r1.exec_time_ns)

    # assemble m rows in msgs-index space; start from original msgs so any
    # scatter entry referencing a row outside the min-plus outputs still
    # matches the reference value
    mfull = msgs.copy()
    for c in range(NCORES):
        lo, hi = c * FPC, (c + 1) * FPC
        mo = np.asarray(r1.results[c]["m_out"]).astype(f32)  # [P, NCH, 2, D]
        mo = mo.transpose(1, 0, 2, 3).reshape(FPAD, 2, D)
        mfull[f2rv_idx[lo:hi]] = mo[:FPC, 0]
        mfull[f2cv_idx[lo:hi]] = mo[:FPC, 1]

    # ---------------- host relay: padded slots ----------------
    # entry t: row mfull[f2v_idx[t]] added to belief[scat[t]]
    order = np.argsort(scat, kind="stable")
    v_sorted = scat[order]
    startv = np.zeros(V_N + 1, np.int64)
    np.cumsum(counts, out=startv[1:])
    rank = np.arange(2 * F_N) - startv[v_sorted]
    slot_rows = mfull[f2v_idx[order]].astype(NPBF)  # [T, D]

    in_maps2 = []
    vmask = np.zeros((P, NW), f32)
    vv = np.arange(VPAD).reshape(NW, P).T  # local v = w*128+p
    vmask[vv < VPC] = 1.0
    iotad = np.broadcast_to(np.arange(D, dtype=f32), (P, D)).copy()
    for c in range(NCORES):
        vlo, vhi = c * VPC, (c + 1) * VPC
        sel = (v_sorted >= vlo) & (v_sorted < vhi)
        posv = pos_of[v_sorted[sel]]
        w = posv // P
        p = posv % P
        k = rank[sel]
        rows = slot_rows[sel]
        mA = w < NA
        mB = ~mA
        slotsA = np.zeros((P, NA, D, KA), NPBF)
        slotsA[p[mA], w[mA], :, k[mA]] = rows[mA]
        slotsB = np.zeros((P, NW - NA, D, KB), NPBF)
        slotsB[p[mB], w[mB] - NA, :, k[mB]] = rows[mB]
        in_maps2.append(dict(slotsA_in=slotsA, slotsB_in=slotsB,
                             vmask_in=vmask, iotad_in=iotad))
    r2 = run_bass_kernel_spmd(k2, in_maps2, core_ids=list(range(NCORES)),
                              trace=trace)
    if r2.exec_time_ns:
        last_exec_times.append(r2.exec_time_ns)

    table = np.zeros((NCORES * VPAD, 16), f32)
    ent_nat = 0.0
    for c in range(NCORES):
        tb = np.asarray(r2.results[c]["table_out"]).astype(f32)  # [P, NW, 16]
        table[c * VPAD:(c + 1) * VPAD] = tb.transpose(1, 0, 2).reshape(VPAD, 16)
        ent_nat += float(np.asarray(r2.results[c]["ent_out"]).sum())

    def vrow(v):  # global v -> table row (count-sorted position per core)
        return (v // VPC) * VPAD + pos_of[v]

    # ---------------- K3: bilinear + cost ----------------
    drv_rows = table[vrow(rv_idx)]  # [F, 16]
    dcv_rows = table[vrow(cv_idx)]
    vr = drv_rows[:, D].astype(np.int64)
    vc = dcv_rows[:, D].astype(np.int64)
    cost_vals = C[np.arange(F_N), vr * D + vc]
    in_maps3 = []
    for c in range(NCORES):
        lo, hi = c * FPC, (c + 1) * FPC
        cvp = np.zeros((FPAD,), f32)
        cvp[:FPC] = cost_vals[lo:hi]
        in_maps3.append(dict(
            c_in=cslices[c],
            drv_in=_to_pcd(drv_rows[lo:hi].astype(NPBF), 16),
            dcv_in=_to_pcd(dcv_rows[lo:hi].astype(NPBF), 16),
            cval_in=np.ascontiguousarray(
                cvp.reshape(NCH, P).T.astype(f32))))
    r3 = run_bass_kernel_spmd(k3, in_maps3, core_ids=list(range(NCORES)),
                              trace=trace)
    if r3.exec_time_ns:
        last_exec_times.append(r3.exec_time_ns)

    per_sum = 0.0
    cost_sum = 0.0
    for c in range(NCORES):
        per_sum += float(np.asarray(r3.results[c]["per_out"]).sum())
        cost_sum += float(np.asarray(r3.results[c]["cost_out"]).sum())

    ent = -ent_nat / np.log(2.0) / V_N
    # f_batch is all zeros; segment_sum into 1 segment then mean == plain sum
    loss = per_sum + 0.1 * ent
    cost_mean = cost_sum
    return np.array([loss, cost_mean], dtype=np.float32)


# revision 96
# speedup vs baseline: 1.0373x; 1.0042x over previous
"""Trainium2 Bass kernel for nn_AttentiveBP (min-plus BP + belief + loss).

Observation: the network's output (loss, cost_mean) depends only on the
min-plus factor updates, the belief scatter-sum, the softmax/entropy, and
the bilinear cost terms. The GAT/GRU/attention subgraph writes msgs[0:2F]
while belief reads msgs[2F:4F], so it is dead code w.r.t. the outputs and
is skipped entirely.

Structure: three SPMD NEFFs over 8 NeuronCores, with host-side index
shuffling (no host arithmetic on the data path):
  K1: stream cost_tensors slice (bf16), compute m_f2rv/m_f2cv (min-plus).
  host: scatter m rows into per-owner padded [v, d, k] slot layout.
  K2: belief = reduce over slots; dist = softmax(-belief); argmax; entropy.
  host: gather dist table rows per factor (rv/cv).
  K3: stream cost_tensors again (bf16); per = sum drv.C.dcv; cost =
      sum C[f, vr, vc] (host element gather, exact fp32).

Perf notes (150.7us fp32 baseline -> ~110us):
  - whole data path in bf16: halves the (serial) DMA stream and enables
    the DVE 2x_1p fast mode (2-byte packed innermost) for TensorTensor.
  - min-reduces replaced by overlapping-halves TT-min trees (min is
    idempotent, so slices [0:8] and [7:15] overlap legally): every tree
    level is a packed 2x TT instead of a no-fast-mode TensorReduce.
  - TRN2 Pool-engine ISA only implements add/mult TensorTensor (no min/
    is_equal, no TensorScalarPtr - walrus rejects them), so Pool absorbs
    the broadcast adds/mults (eff 0.42) while all min/compare work stays
    on the DVE. K1 runs both engines at 100% occupancy.
  - K3's dot-product accumulation runs on the idle Activation engine
    (activation accum_out); the C*o product is a packed 2x TT; o
    alternates DVE/Pool in half-tiles.
  - all input loads share one DMA queue so the serial DMA device follows
    an exact priority order; first/last tiles are split in half to cut
    pipeline fill/drain; cross-engine consumers are emitted 1-2 tiles
    behind their producers to avoid wait-queue head-of-line blocking.
  - K2: two interleaved 15-window lanes; entropy via the identity
    sum dist*ln(dist) = -sum dist*bel - ln(den) (removes the big Ln ops);
    two-tier slot layout (host sorts each core's variables by slot count:
    8 windows at depth KA=max-count, 22 at depth KB=tier-B max) cuts the
    padded belief-slot traffic and tree work ~40%.
"""
import os
import sys

sys.path.insert(0, "/opt/trn_rl_repo")

import ml_dtypes
import numpy as np

import concourse.bass as bass
import concourse.bacc as bacc
import concourse.tile as tile
from concourse import mybir
from concourse.bass_utils import run_bass_kernel_spmd

F_N = 100000
V_N = 30000
D = 15
NCORES = 8
FPC = F_N // NCORES          # 12500 factors per core
P = 128
NCH = (FPC + P - 1) // P     # 98 chunks of 128 factors
FPAD = NCH * P               # 12544 padded factors per core
G = 14                       # chunks per compute tile
NTILE = NCH // G             # 7 tiles exactly
VPC = V_N // NCORES          # 3750 v per core
NW = (VPC + P - 1) // P      # 30 windows
VPAD = NW * P                # 3840

FP32 = mybir.dt.float32
BF16 = mybir.dt.bfloat16
I32 = mybir.dt.int32
AX = mybir.AxisListType
OP = mybir.AluOpType
ACT = mybir.ActivationFunctionType
NPBF = ml_dtypes.bfloat16

last_exec_times = []

_cache = {}


def _min_tree(nc, pool, src, g, axis, tag):
    """min over one D-axis of src[:, :g, D, D] via overlapping-halves TT-min.

    axis=3: min over innermost j -> [P, g, D]; axis=2: min over i.
    Overlap ([0:8] vs [7:15]) is legal because min is idempotent.
    All levels run on the DVE at packed-bf16 2x (the Pool engine's ISA
    only implements add/mult TensorTensor, so min cannot offload).
    """
    if axis == 3:
        shapes = [(D, 8), (D, 4), (D, 2)]
    else:
        shapes = [(8, D), (4, D), (2, D)]
    cur = src
    n = 15
    for lvl, shp in enumerate(shapes):
        nxt = pool.tile([P, G, shp[0], shp[1]], BF16, tag=f"{tag}{lvl}")
        h = (n + 1) // 2
        if axis == 3:
            nc.vector.tensor_tensor(out=nxt[:, :g], in0=cur[:, :g, :, 0:h],
                                    in1=cur[:, :g, :, n - h:n], op=OP.min)
        else:
            nc.vector.tensor_tensor(out=nxt[:, :g], in0=cur[:, :g, 0:h, :],
                                    in1=cur[:, :g, n - h:n, :], op=OP.min)
        cur = nxt
        n = h
    return cur  # [P, G, D, 2] or [P, G, 2, D]


def _build_k1():
    nc = bacc.Bacc(None)
    c_in = nc.dram_tensor("c_in", [P, NCH, D * D], BF16, kind="ExternalInput")
    mrv_in = nc.dram_tensor("mrv_in", [P, NCH, D], BF16, kind="ExternalInput")
    mcv_in = nc.dram_tensor("mcv_in", [P, NCH, D], BF16, kind="ExternalInput")
    # merged [m1 | m2] rows: m_out[p, c, 0, :] = m1, m_out[p, c, 1, :] = m2
    m_out = nc.dram_tensor("m_out", [P, NCH, 2, D], BF16, kind="ExternalOutput")

    H = NCH // 2
    with tile.TileContext(nc) as tc:
        with tc.tile_pool(name="cts", bufs=4) as cpool, \
             tc.tile_pool(name="scr", bufs=4) as spool, \
             tc.tile_pool(name="tre", bufs=2) as tpool, \
             tc.tile_pool(name="mout", bufs=4) as mpool, \
             tc.tile_pool(name="msgs", bufs=1) as gpool:
            # single sync queue => serial DMA order is exactly: msg first
            # halves, C tile 0 (in halves, so the first half-tile's compute
            # starts ~4.3us in), C tile 1, msg second halves, then the
            # loop's C tiles. Outputs go on the scalar queue.
            G2 = G // 2
            mrv = gpool.tile([P, NCH, D], BF16)
            mcv = gpool.tile([P, NCH, D], BF16)
            Q = H // 2 + 1
            nc.sync.dma_start(out=mrv[:, 0:Q], in_=mrv_in[:, 0:Q])
            nc.sync.dma_start(out=mcv[:, 0:Q], in_=mcv_in[:, 0:Q])
            ct0 = cpool.tile([P, G, D * D], BF16, tag="ct")
            nc.sync.dma_start(out=ct0[:, 0:G2], in_=c_in[:, 0:G2, :])
            nc.sync.dma_start(out=ct0[:, G2:G], in_=c_in[:, G2:G, :])
            nc.sync.dma_start(out=mrv[:, Q:H], in_=mrv_in[:, Q:H])
            nc.sync.dma_start(out=mcv[:, Q:H], in_=mcv_in[:, Q:H])
            ct1 = cpool.tile([P, G, D * D], BF16, tag="ct")
            nc.sync.dma_start(out=ct1[:], in_=c_in[:, G:2 * G, :])
            nc.sync.dma_start(out=mrv[:, H:NCH], in_=mrv_in[:, H:NCH])
            nc.sync.dma_start(out=mcv[:, H:NCH], in_=mcv_in[:, H:NCH])

            # Pool computes s2 = C + mrv for all but the last two half-tiles
            # (its add is 3.7x slower than a packed DVE add, but it's the
            # only engine that can absorb elementwise work). The m2 tree
            # that consumes s2 runs TWO tiles behind so the DVE wait queue
            # never head-of-line blocks on the slow Pool op. First and last
            # 14-chunk tiles are split in half to shorten startup and drain.
            G2 = G // 2
            tiles = [(0, G2, "P"), (G2, G2, "P")]
            for k in range(1, NTILE - 1):
                tiles.append((k * G, G, "P"))
            tiles += [(NCH - G, G2, "D"), (NCH - G2, G2, "D")]

            pend = []  # [(s2, mo, g0, g), ...]
            NT = len(tiles)
            ctz = None
            for i in range(NT + 2):
                if i < NT:
                    g0, g, eng = tiles[i]
                    if i <= 1:
                        ct, cta = ct0, g0
                    elif i == 2:
                        ct, cta = ct1, 0
                    elif i >= NT - 2:
                        if ctz is None:
                            ctz = cpool.tile([P, G, D * D], BF16, tag="ct")
                            nc.sync.dma_start(out=ctz[:, 0:G2],
                                              in_=c_in[:, NCH - G:NCH - G2, :])
                            nc.sync.dma_start(out=ctz[:, G2:G],
                                              in_=c_in[:, NCH - G2:NCH, :])
                        ct, cta = ctz, g0 - (NCH - G)
                    else:
                        ct = cpool.tile([P, G, D * D], BF16, tag="ct")
                        nc.sync.dma_start(out=ct[:, :g, :],
                                          in_=c_in[:, g0:g0 + g, :])
                        cta = 0
                    ctv = ct[:, cta:cta + g, :].rearrange(
                        "p g (i j) -> p g i j", i=D)
                    mo = mpool.tile([P, G, 2, D], BF16, tag="mo")

                    # s2 = C + mrv bcast over j. Pool tiles read the
                    # broadcast AP directly; on the DVE tiles the idle ACT
                    # engine first expands mrv to a packed tile so the add
                    # runs at 2x instead of broadcast-1x.
                    s2 = spool.tile([P, G, D, D], BF16, tag="s2")
                    mrv_b = bass.AP(tensor=mrv.tensor,
                                    offset=mrv.offset + g0 * D,
                                    ap=[mrv.ap[0], [D, g], [1, D], [0, D]])
                    if eng == "P":
                        nc.gpsimd.tensor_tensor(out=s2[:, :g], in0=ctv,
                                                in1=mrv_b, op=OP.add)
                    else:
                        mrvx = spool.tile([P, G, D, D], BF16, tag="mrvx")
                        nc.scalar.activation(out=mrvx[:, :g], in_=mrv_b,
                                             func=ACT.Copy)
                        nc.vector.tensor_tensor(out=s2[:, :g], in0=ctv,
                                                in1=mrvx[:, :g], op=OP.add)

                    # s1 = C + mcv bcast over i (packed innermost -> DVE 2x)
                    s1 = spool.tile([P, G, D, D], BF16, tag="s1")
                    mcv_b = bass.AP(tensor=mcv.tensor,
                                    offset=mcv.offset + g0 * D,
                                    ap=[mcv.ap[0], [D, g], [0, D], [1, D]])
                    nc.vector.tensor_tensor(out=s1[:, :g], in0=ctv, in1=mcv_b,
                                            op=OP.add)
                    # m1 = min_j s1 (tree, same-engine producer: no stall)
                    t1 = _min_tree(nc, tpool, s1, g, 3, "t1")
                    i0 = bass.AP(tensor=t1.tensor, offset=t1.offset,
                                 ap=[t1.ap[0], [2 * D, g], [2, D]])
                    i1 = bass.AP(tensor=t1.tensor, offset=t1.offset + 1,
                                 ap=[t1.ap[0], [2 * D, g], [2, D]])
                    m1o = bass.AP(tensor=mo.tensor, offset=mo.offset,
                                  ap=[mo.ap[0], [2 * D, g], [1, D]])
                    nc.vector.tensor_tensor(out=m1o, in0=i0, in1=i1, op=OP.min)

                def emit_t2(ps2, pmo, pg0, pg):
                    # m2 = min_i s2 (tree)
                    t2 = _min_tree(nc, tpool, ps2, pg, 2, "t2")
                    j0 = bass.AP(tensor=t2.tensor, offset=t2.offset,
                                 ap=[t2.ap[0], [2 * D, pg], [1, D]])
                    j1 = bass.AP(tensor=t2.tensor, offset=t2.offset + D,
                                 ap=[t2.ap[0], [2 * D, pg], [1, D]])
                    m2o = bass.AP(tensor=pmo.tensor, offset=pmo.offset + D,
                                  ap=[pmo.ap[0], [2 * D, pg], [1, D]])
                    nc.vector.tensor_tensor(out=m2o, in0=j0, in1=j1, op=OP.min)
                    nc.scalar.dma_start(out=m_out[:, pg0:pg0 + pg],
                                        in_=pmo[:, :pg])

                if len(pend) == 2 or (i >= NT and pend):
                    emit_t2(*pend.pop(0))
                if i < NT:
                    if eng == "D":
                        # DVE-produced s2: same-engine in-order, no lag
                        # needed -- shortens the drain on the final tiles
                        emit_t2(s2, mo, g0, g)
                    else:
                        pend.append((s2, mo, g0, g))
    nc.compile()
    return nc


def _build_k2(KA, KB):
    WG = 15   # windows per lane (2 interleaved lanes)
    NA = 8    # tier-A windows (high-count variables, K = KA)
    NB = NW - NA
    NBG = NB // 2
    nc = bacc.Bacc(None)
    slotsA_in = nc.dram_tensor("slotsA_in", [P, NA, D, KA], BF16,
                               kind="ExternalInput")
    slotsB_in = nc.dram_tensor("slotsB_in", [P, NB, D, KB], BF16,
                               kind="ExternalInput")
    vmask_in = nc.dram_tensor("vmask_in", [P, NW], FP32, kind="ExternalInput")
    iotad_in = nc.dram_tensor("iotad_in", [P, D], FP32, kind="ExternalInput")
    table_out = nc.dram_tensor("table_out", [P, NW, 16], BF16,
                               kind="ExternalOutput")
    ent_out = nc.dram_tensor("ent_out", [P, NW // WG], FP32,
                             kind="ExternalOutput")
    with tile.TileContext(nc) as tc:
        with tc.tile_pool(name="sl", bufs=2) as slp, \
             tc.tile_pool(name="sb", bufs=1) as sb:
            vmask = sb.tile([P, NW], FP32)
            nc.scalar.dma_start(out=vmask[:], in_=vmask_in[:])
            iotad = sb.tile([P, D], FP32)
            nc.scalar.dma_start(out=iotad[:], in_=iotad_in[:])
            biast = sb.tile([P, 1], FP32)
            nc.vector.memset(biast[:], 1e-6)

            # belief[p, w, d] = sum_k slots[p, w, d, k] (packed 2x add tree,
            # fp32 TensorReduce tail). Host sorts each core's variables by
            # slot count: windows 0..NA-1 use depth KA, the rest depth KB.
            bel = sb.tile([P, NW, D], FP32)

            def bel_group(dram, woff, w0, nw, K, tag):
                h1 = K // 2
                h2 = h1 // 2
                sl = slp.tile([P, nw, D, K], BF16, tag=f"sl{tag}")
                hh = (nw + 1) // 2
                nc.sync.dma_start(out=sl[:, 0:hh],
                                  in_=dram[:, woff:woff + hh])
                nc.sync.dma_start(out=sl[:, hh:nw],
                                  in_=dram[:, woff + hh:woff + nw])
                a = slp.tile([P, nw, D, h1], BF16, tag=f"a{tag}")
                nc.vector.tensor_tensor(out=a[:], in0=sl[:, :, :, 0:h1],
                                        in1=sl[:, :, :, h1:K], op=OP.add)
                b = slp.tile([P, nw, D, h2], BF16, tag=f"b{tag}")
                nc.vector.tensor_tensor(out=b[:], in0=a[:, :, :, 0:h2],
                                        in1=a[:, :, :, h2:h1], op=OP.add)
                nc.vector.tensor_reduce(out=bel[:, w0:w0 + nw], in_=b[:],
                                        axis=AX.X, op=OP.add)

            bel_group(slotsA_in, 0, 0, NA, KA, "A")
            bel_group(slotsB_in, 0, NA, NBG, KB, "B0")
            bel_group(slotsB_in, NBG, NA + NBG, NB - NBG, KB, "B1")

            # two lanes of WG windows run the softmax/argmax/entropy chain
            # interleaved so engine idle time overlaps across lanes.
            e = sb.tile([P, NW, D], FP32)
            den = sb.tile([P, NW], FP32)
            rden = sb.tile([P, NW], FP32)
            dist = sb.tile([P, NW, D], FP32)
            dtb = sb.tile([P, NW, D], FP32)
            mx = sb.tile([P, NW], FP32)
            ohm = sb.tile([P, NW, D], FP32)
            tmp = sb.tile([P, NW, D], FP32)
            amax = sb.tile([P, NW], FP32)
            lnd = sb.tile([P, NW, D], FP32)
            integ = sb.tile([P, NW, D], FP32)
            dead = sb.tile([P, NW, D], FP32)
            entp = sb.tile([P, NW // WG], FP32)
            lnjunk = sb.tile([P, 1], FP32)
            tbl = sb.tile([P, NW, 16], BF16)

            LS = [slice(w0, w0 + WG) for w0 in range(0, NW, WG)]
            iota_b = bass.AP(tensor=iotad.tensor, offset=iotad.offset,
                             ap=[iotad.ap[0], [0, WG], [1, D]])

            def bcast(tile2, s):  # [P, NW] col-slice -> bcast over D
                return bass.AP(tensor=tile2.tensor,
                               offset=tile2.offset + s.start,
                               ap=[tile2.ap[0], [1, WG], [0, D]])

            # argmax(dist) == argmin(bel) exactly (exp is monotone, den
            # shared per row), so the argmax chain starts straight from bel
            # and overlaps the softmax chain instead of following it.
            # eps=1e-4 per index step is ~52 ulp at |bel|~13, so perturbed
            # values are strictly distinct and exactly one index matches.
            for s in LS:
                nc.vector.scalar_tensor_tensor(out=dtb[:, s], in0=iota_b,
                                               scalar=1e-4, in1=bel[:, s],
                                               op0=OP.mult, op1=OP.add)
            for s in LS:
                nc.vector.tensor_reduce(out=mx[:, s], in_=dtb[:, s], axis=AX.X,
                                        op=OP.min)
            for s in LS:
                nc.scalar.activation(out=e[:, s], in_=bel[:, s], func=ACT.Exp,
                                     scale=-1.0)
            # dummy Ln pulls the Ln act-table load off the critical path
            nc.scalar.activation(out=lnjunk[:], in_=biast[:], func=ACT.Ln,
                                 bias=biast[:, 0:1])
            for s in LS:
                nc.vector.tensor_reduce(out=den[:, s], in_=e[:, s], axis=AX.X,
                                        op=OP.add)
            for s in LS:
                nc.vector.reciprocal(out=rden[:, s], in_=den[:, s])
            for s in LS:
                nc.vector.tensor_tensor(out=dist[:, s], in0=e[:, s],
                                        in1=bcast(rden, s), op=OP.mult)
            # entropy identity: sum_d dist*ln(dist) = sum_d dist*(-bel)
            # - ln(den)  (since dist = exp(-bel)/den and sum_d dist = 1);
            # the +1e-6 epsilon in the reference shifts the sum by O(1e-5)
            # relative -- far below tolerance. Only a tiny [P, NW] Ln needed.
            # One Ln op over BOTH lanes: its data dependency (den of lane 1)
            # forces it after both Exps, so the act-func table loads exactly
            # twice (Exp set, then Ln set) instead of thrashing.
            lnden = sb.tile([P, NW], FP32)
            for s in LS:
                nc.scalar.activation(out=lnden[:, s], in_=den[:, s],
                                     func=ACT.Ln, bias=biast[:, 0:1])
            for s in LS:
                nc.vector.scalar_tensor_tensor(out=lnd[:, s], in0=bel[:, s],
                                               scalar=-1.0, in1=bcast(lnden, s),
                                               op0=OP.mult, op1=OP.subtract)
            for s in LS:
                nc.vector.tensor_tensor(out=ohm[:, s], in0=dtb[:, s],
                                        in1=bcast(mx, s), op=OP.is_equal)
            for s in LS:
                nc.gpsimd.tensor_tensor(out=tmp[:, s], in0=ohm[:, s],
                                        in1=iota_b, op=OP.mult)
            for s in LS:
                nc.vector.tensor_reduce(out=amax[:, s], in_=tmp[:, s],
                                        axis=AX.X, op=OP.add)
            for s in LS:
                nc.gpsimd.tensor_tensor(out=integ[:, s], in0=lnd[:, s],
                                        in1=dist[:, s], op=OP.mult)
            for li, s in enumerate(LS):
                mask_b = bass.AP(tensor=vmask.tensor,
                                 offset=vmask.offset + s.start,
                                 ap=[vmask.ap[0], [1, WG], [0, D]])
                nc.vector.scalar_tensor_tensor(out=dead[:, s], in0=integ[:, s],
                                               scalar=1.0, in1=mask_b,
                                               op0=OP.mult, op1=OP.mult,
                                               accum_out=entp[:, li:li + 1])
            for s in LS:
                nc.vector.tensor_copy(out=tbl[:, s, 0:D], in_=dist[:, s])
            amax3 = bass.AP(tensor=amax.tensor, offset=amax.offset,
                            ap=[amax.ap[0], amax.ap[1], [1, 1]])
            nc.vector.tensor_copy(out=tbl[:, :, D:D + 1], in_=amax3)
            nc.sync.dma_start(out=table_out[:], in_=tbl[:])
            nc.scalar.dma_start(out=ent_out[:], in_=entp[:])
    nc.compile()
    return nc


def _build_k3():
    nc = bacc.Bacc(None)
    c_in = nc.dram_tensor("c_in", [P, NCH, D * D], BF16, kind="ExternalInput")
    drv_in = nc.dram_tensor("drv_in", [P, NCH, 16], BF16, kind="ExternalInput")
    dcv_in = nc.dram_tensor("dcv_in", [P, NCH, 16], BF16, kind="ExternalInput")
    cval_in = nc.dram_tensor("cval_in", [P, NCH], FP32, kind="ExternalInput")
    per_out = nc.dram_tensor("per_out", [P, 1], FP32, kind="ExternalOutput")
    cost_out = nc.dram_tensor("cost_out", [P, 1], FP32, kind="ExternalOutput")

    # per-tile engine split balancing DVE ~ gpsimd ~ ACT busy time:
    # o = drv (x) dcv on gpsimd for GP_TILES (DVE otherwise); the sum runs
    # on ACT (accum_out) except POOL_SUM tiles.
    H = NCH // 2
    with tile.TileContext(nc) as tc:
        with tc.tile_pool(name="cts", bufs=4) as cpool, \
             tc.tile_pool(name="scr", bufs=4) as spool, \
             tc.tile_pool(name="sb", bufs=1) as sb:
            drv = sb.tile([P, NCH, 16], BF16)
            dcv = sb.tile([P, NCH, 16], BF16)
            cvals = sb.tile([P, NCH], FP32)
            perC = sb.tile([P, 2 * NTILE], FP32)
            # all input loads share the sync queue so their serial order on
            # the (single) DMA device is exactly this priority order: first
            # o/prod inputs, then C tiles interleaved with the second halves.
            nc.sync.dma_start(out=drv[:, 0:H], in_=drv_in[:, 0:H])
            nc.sync.dma_start(out=dcv[:, 0:H], in_=dcv_in[:, 0:H])
            cts = []
            for _ci in range(3):
                ct_pre = cpool.tile([P, G, D * D], BF16, tag="ct")
                cts.append(ct_pre)
            # tile-0 C load split in half so the first prod gates on a
            # half-size transfer
            G2 = G // 2
            nc.sync.dma_start(out=cts[0][:, 0:G2], in_=c_in[:, 0:G2, :])
            nc.sync.dma_start(out=cts[0][:, G2:G], in_=c_in[:, G2:G, :])
            nc.sync.dma_start(out=cts[1][:], in_=c_in[:, G:2 * G, :])
            nc.sync.dma_start(out=drv[:, H:NCH], in_=drv_in[:, H:NCH])
            nc.sync.dma_start(out=dcv[:, H:NCH], in_=dcv_in[:, H:NCH])
            nc.sync.dma_start(out=cts[2][:], in_=c_in[:, 2 * G:3 * G, :])
            nc.sync.dma_start(out=cvals[:], in_=cval_in[:])

            # prod/sum for DVE-side o tiles follow immediately (same-engine
            # in-order). For Pool-side o tiles they are emitted two tiles
            # later so the slow Pool mult never head-of-line blocks the DVE
            # wait queue. perC columns are independent, so tile sums may
            # complete out of order.
            def emit_prod_sum(pct, pcta, po, pg, col):
                pctv = pct[:, pcta:pcta + pg, :].rearrange(
                    "p g (i j) -> p g i j", i=D)
                prod = spool.tile([P, G, D, D], BF16, tag="prod")
                nc.vector.tensor_tensor(out=prod[:, :pg], in0=pctv,
                                        in1=po[:, :pg], op=OP.mult)
                junk = spool.tile([P, G, D, D], BF16, tag="junka")
                if col == 2 * NTILE - 1:
                    # the ACT queue lags supply by the end; the final sum
                    # runs on the then-idle DVE instead (max(x,x) = x, so
                    # the STT out is exact and accum_out gives the sum)
                    nc.vector.scalar_tensor_tensor(
                        out=junk[:, :pg], in0=prod[:, :pg], scalar=1.0,
                        in1=prod[:, :pg], op0=OP.mult, op1=OP.max,
                        accum_out=perC[:, col:col + 1])
                else:
                    nc.scalar.activation(out=junk[:, :pg], in_=prod[:, :pg],
                                         func=ACT.Identity,
                                         accum_out=perC[:, col:col + 1])

            # all tiles split into 7-chunk halves: finer granularity lets the
            # three-engine o -> prod -> sum pipeline pack much tighter.
            # Pool takes every other o (starting late, after the DVE pair).
            tiles = []
            for hh in range(2 * NTILE):
                kind = "D" if hh == 0 or hh % 2 == 0 else "P"
                tiles.append((hh * (G // 2), G // 2, kind))

            # uniform one-tile lag: prod/sum of tiles[i-1] are emitted during
            # tile i
            pend = None
            cur_ct = None
            for i, (g0, g, kind) in enumerate(tiles):
                pair, cta = divmod(g0, G)
                if pair < 3:
                    ct = cts[pair]
                elif cta == 0:
                    cur_ct = cpool.tile([P, G, D * D], BF16, tag="ct")
                    nc.sync.dma_start(out=cur_ct[:],
                                      in_=c_in[:, g0:g0 + G, :])
                    ct = cur_ct
                else:
                    ct = cur_ct

                o = spool.tile([P, G, D, D], BF16, tag="o")

                def drdc(a0, n):
                    db = bass.AP(tensor=drv.tensor,
                                 offset=drv.offset + (g0 + a0) * 16,
                                 ap=[drv.ap[0], [16, n], [1, D], [0, D]])
                    cb = bass.AP(tensor=dcv.tensor,
                                 offset=dcv.offset + (g0 + a0) * 16,
                                 ap=[dcv.ap[0], [16, n], [0, D], [1, D]])
                    return db, cb

                if kind == "P":
                    db, cb = drdc(0, g)
                    nc.gpsimd.tensor_tensor(out=o[:, :g], in0=db, in1=cb,
                                            op=OP.mult)
                elif kind == "S":
                    h = g // 2
                    db, cb = drdc(0, h)
                    nc.vector.tensor_tensor(out=o[:, :h], in0=db, in1=cb,
                                            op=OP.mult)
                    db, cb = drdc(h, g - h)
                    nc.gpsimd.tensor_tensor(out=o[:, h:g], in0=db,
                                            in1=cb, op=OP.mult)
                else:
                    db, cb = drdc(0, g)
                    nc.vector.tensor_tensor(out=o[:, :g], in0=db, in1=cb,
                                            op=OP.mult)
                if pend is not None:
                    emit_prod_sum(*pend)
                pend = (ct, cta, o, g, i)
            emit_prod_sum(*pend)
            costp = sb.tile([P, 1], FP32)
            nc.vector.tensor_reduce(out=costp[:], in_=cvals[:], axis=AX.X,
                                    op=OP.add)
            nc.sync.dma_start(out=cost_out[:], in_=costp[:])
            perp = sb.tile([P, 1], FP32)
            nc.vector.tensor_reduce(out=perp[:], in_=perC[:], axis=AX.X,
                                    op=OP.add)
            nc.sync.dma_start(out=per_out[:], in_=perp[:])
    nc.compile()
    return nc


def _get_programs(KA, KB):
    key = ("k", KA, KB)
    if key not in _cache:
        _cache[key] = (_build_k1(), _build_k2(KA, KB), _build_k3())
    return _cache[key]


def _to_pcd(a, width):
    """[FPC(+), width] f32/bf16 -> [P, NCH, width] bf16, factor = c*128+p."""
    out = np.zeros((FPAD, width), NPBF)
    out[:a.shape[0]] = a
    return np.ascontiguousarray(out.reshape(NCH, P, width).transpose(1, 0, 2))


def kernel(**inp):
    global last_exec_times
    last_exec_times = []
    f32 = np.float32

    msgs = np.asarray(inp["msgs"], f32)
    C = np.ascontiguousarray(np.asarray(inp["cost_tensors"], f32).reshape(F_N, D * D))
    rv2f_idx = np.asarray(inp["msg_rv2f_idxes"], np.int64)
    cv2f_idx = np.asarray(inp["msg_cv2f_idxes"], np.int64)
    f2rv_idx = np.asarray(inp["msg_f2rv_idxes"], np.int64)
    f2cv_idx = np.asarray(inp["msg_f2cv_idxes"], np.int64)
    f2v_idx = np.asarray(inp["msg_f2v_per_v_idxes"], np.int64)
    scat = np.asarray(inp["f2v_per_v_scatter_idxes"], np.int64)
    rv_idx = np.asarray(inp["rv_idxes"], np.int64)
    cv_idx = np.asarray(inp["cv_idxes"], np.int64)

    m_rv2f = msgs[rv2f_idx]   # [F, D]
    m_cv2f = msgs[cv2f_idx]

    trace = bool(int(os.environ.get("KERNEL_TRACE", "0")))

    # --- slot depths from the actual scatter. Variables are sorted per
    # core by slot count (descending); the NA*128 highest-count ones go to
    # tier-A windows (depth KA = global max), the rest to tier-B windows
    # (depth KB = max tier-B count). Correct for any distribution by
    # construction. ---
    NA = 8
    counts = np.bincount(scat, minlength=V_N)
    KA = max(int(counts.max()), 4)
    KA = ((KA + 3) // 4) * 4
    pos_of = np.empty(V_N, np.int64)  # per-core count-sorted position
    kb = 1
    for c in range(NCORES):
        vlo, vhi = c * VPC, (c + 1) * VPC
        cc = counts[vlo:vhi]
        ordv = np.argsort(-cc, kind="stable")
        pos = np.empty(VPC, np.int64)
        pos[ordv] = np.arange(VPC)
        pos_of[vlo:vhi] = pos
        if VPC > NA * P:
            kb = max(kb, int(cc[ordv[NA * P:]].max()))
    KB = min(((kb + 3) // 4) * 4, KA)
    k1, k2, k3 = _get_programs(KA, KB)

    # ---------------- K1: min-plus ----------------
    Cb = C.astype(NPBF)
    in_maps1 = []
    cslices = []
    for c in range(NCORES):
        lo, hi = c * FPC, (c + 1) * FPC
        cs = _to_pcd(Cb[lo:hi], D * D)
        cslices.append(cs)
        in_maps1.append(dict(c_in=cs,
                             mrv_in=_to_pcd(m_rv2f[lo:hi], D),
                             mcv_in=_to_pcd(m_cv2f[lo:hi], D)))
    r1 = run_bass_kernel_spmd(k1, in_maps1, core_ids=list(range(NCORES)),
                              trace=trace)
    if r1.exec_time_ns:
        last_exec_times.append(r1.exec_time_ns)

    # assemble m rows in msgs-index space; start from original msgs so any
    # scatter entry referencing a row outside the min-plus outputs still
    # matches the reference value
    mfull = msgs.copy()
    for c in range(NCORES):
        lo, hi = c * FPC, (c + 1) * FPC
        mo = np.asarray(r1.results[c]["m_out"]).astype(f32)  # [P, NCH, 2, D]
        mo = mo.transpose(1, 0, 2, 3).reshape(FPAD, 2, D)
        mfull[f2rv_idx[lo:hi]] = mo[:FPC, 0]
        mfull[f2cv_idx[lo:hi]] = mo[:FPC, 1]

    # ---------------- host relay: padded slots ----------------
    # entry t: row mfull[f2v_idx[t]] added to belief[scat[t]]
    order = np.argsort(scat, kind="stable")
    v_sorted = scat[order]
    startv = np.zeros(V_N + 1, np.int64)
    np.cumsum(counts, out=startv[1:])
    rank = np.arange(2 * F_N) - startv[v_sorted]
    slot_rows = mfull[f2v_idx[order]].astype(NPBF)  # [T, D]

    in_maps2 = []
    vmask = np.zeros((P, NW), f32)
    vv = np.arange(VPAD).reshape(NW, P).T  # local v = w*128+p
    vmask[vv < VPC] = 1.0
    iotad = np.broadcast_to(np.arange(D, dtype=f32), (P, D)).copy()
    for c in range(NCORES):
        vlo, vhi = c * VPC, (c + 1) * VPC
        sel = (v_sorted >= vlo) & (v_sorted < vhi)
        posv = pos_of[v_sorted[sel]]
        w = posv // P
        p = posv % P
        k = rank[sel]
        rows = slot_rows[sel]
        mA = w < NA
        mB = ~mA
        slotsA = np.zeros((P, NA, D, KA), NPBF)
        slotsA[p[mA], w[mA], :, k[mA]] = rows[mA]
        slotsB = np.zeros((P, NW - NA, D, KB), NPBF)
        slotsB[p[mB], w[mB] - NA, :, k[mB]] = rows[mB]
        in_maps2.append(dict(slotsA_in=slotsA, slotsB_in=slotsB,
                             vmask_in=vmask, iotad_in=iotad))
    r2 = run_bass_kernel_spmd(k2, in_maps2, core_ids=list(range(NCORES)),
                              trace=trace)
    if r2.exec_time_ns:
        last_exec_times.append(r2.exec_time_ns)

    table = np.zeros((NCORES * VPAD, 16), f32)
    ent_nat = 0.0
    for c in range(NCORES):
        tb = np.asarray(r2.results[c]["table_out"]).astype(f32)  # [P, NW, 16]
        table[c * VPAD:(c + 1) * VPAD] = tb.transpose(1, 0, 2).reshape(VPAD, 16)
        ent_nat += float(np.asarray(r2.results[c]["ent_out"]).sum())

    def vrow(v):  # global v -> table row (count-sorted position per core)
        return (v // VPC) * VPAD + pos_of[v]

    # ---------------- K3: bilinear + cost ----------------
    drv_rows = table[vrow(rv_idx)]  # [F, 16]
    dcv_rows = table[vrow(cv_idx)]
    vr = np.clip(drv_rows[:, D].astype(np.int64), 0, D - 1)
    vc = np.clip(dcv_rows[:, D].astype(np.int64), 0, D - 1)
    cost_vals = C[np.arange(F_N), vr * D + vc]
    in_maps3 = []
    for c in range(NCORES):
        lo, hi = c * FPC, (c + 1) * FPC
        cvp = np.zeros((FPAD,), f32)
        cvp[:FPC] = cost_vals[lo:hi]
        in_maps3.append(dict(
            c_in=cslices[c],
            drv_in=_to_pcd(drv_rows[lo:hi].astype(NPBF), 16),
            dcv_in=_to_pcd(dcv_rows[lo:hi].astype(NPBF), 16),
            cval_in=np.ascontiguousarray(
                cvp.reshape(NCH, P).T.astype(f32))))
    r3 = run_bass_kernel_spmd(k3, in_maps3, core_ids=list(range(NCORES)),
                              trace=trace)
    if r3.exec_time_ns:
        last_exec_times.append(r3.exec_time_ns)

    per_sum = 0.0
    cost_sum = 0.0
    for c in range(NCORES):
        per_sum += float(np.asarray(r3.results[c]["per_out"]).sum())
        cost_sum += float(np.asarray(r3.results[c]["cost_out"]).sum())

    ent = -ent_nat / np.log(2.0) / V_N
    # f_batch is all zeros; segment_sum into 1 segment then mean == plain sum
    loss = per_sum + 0.1 * ent
    cost_mean = cost_sum
    return np.array([loss, cost_mean], dtype=np.float32)
